# revision 37
# baseline (speedup 1.0000x reference)
"""BiAttention Trainium2 Bass kernel (v4 — fp16 streaming pipeline).

Per-core (one batch per NeuronCore, batch=8 over 8 cores):
  att[i,j] = input_dot[i] + memory_dot[j] + (input*dot_scale) @ memory^T - NEG*(1-mask[j])
  weight_one = softmax_j(att);  output_one = weight_one @ memory
  weight_two = softmax_i(max_j att);  output_two = weight_two @ input
  out = concat([input, output_one, input*output_one, output_two*output_one], -1)

Implementation notes:
  - input_dot cancels in softmax_j; only memory_dot + mask pad enter the bias.
  - Unmasked memory rows are permuted first host-side; only Lmp rows computed.
  - Scores built transposed (S^T[j,i]): per-j bias is a per-partition ACT bias,
    exp lands in the P^T layout phase 2 needs.  C = max(mvec) global shift.
  - Everything on-chip is fp16 (PT, operands, outputs); fp32 for bias/psum/stats.
  - dot_scale folded into x^T on device (per-partition scalar over d).
  - max_j att recovered from running max of exp tiles (M1) + PE transposes.
  - denominator from an appended ones-column in maug.
  - Device writes only the 3 computed output blocks in fp16; the `input` block
    is assembled host-side (pure copy).  Host prep is layout/dtype marshalling
    only — all arithmetic happens on device.
  - PE warmup matmuls during the load phase un-throttle the HAM clock gate.
  - Loads prioritized: mt first alone, so phase-1 matmuls start earliest.
"""

import math
import numpy as np

import concourse.bass as bass
import concourse.mybir as mybir
import concourse.tile as tile
import concourse.bacc as bacc
from concourse import bass_isa
from concourse.bass_utils import run_bass_kernel_spmd

F32 = mybir.dt.float32
F16 = mybir.dt.float16
AX = mybir.AxisListType
ALU = mybir.AluOpType
ACTF = mybir.ActivationFunctionType

N_CORES = 8
NEG = 1e30

_NC_CACHE: dict = {}
LAST_RESULTS = None  # BassKernelResults of the most recent run (for test harness)


def build_nc(Li: int, Lmp: int, d: int):
    """Build the single-core SPMD program.  Li, d fixed; Lmp = padded #unmasked."""
    assert Li % 128 == 0 and Lmp % 128 == 0 and d == 256
    NI = Li // 128          # i tiles (16)
    NJ = Lmp // 128         # j tiles (9)
    D1 = d + 1
    H = 1024                # phase-1 i-chunk
    NH = Li // H            # 2
    TPH = H // 128          # i tiles per chunk (8)
    QT = 4                  # i tiles per output store chunk

    nc = bacc.Bacc("TRN2", target_bir_lowering=False, debug=False,
                   num_devices=N_CORES)

    xt_d = nc.dram_tensor("xt", [128, 2 * Li], F16, kind="ExternalInput")
    xb_d = nc.dram_tensor("xb", [128, NI * d], F16, kind="ExternalInput")
    mt_d = nc.dram_tensor("mt", [128, 2 * Lmp], F16, kind="ExternalInput")
    maug_d = nc.dram_tensor("maug", [128, NJ * D1], F16, kind="ExternalInput")
    mp_d = nc.dram_tensor("mp", [128, NJ], F32, kind="ExternalInput")
    dsc_d = nc.dram_tensor("dsc", [128, 2], F32, kind="ExternalInput")
    win_d = nc.dram_tensor("winb", [128, d], F16, kind="ExternalInput")
    wmemc_d = nc.dram_tensor("wmemc", [128, 2], F16, kind="ExternalInput")
    id_d = nc.dram_tensor("id16", [128, 128], F16, kind="ExternalInput")
    out_d = nc.dram_tensor("out", [Li, 3 * d], F16, kind="ExternalOutput")

    with tile.TileContext(nc) as tc:
        with (
            tc.tile_pool(name="singles", bufs=1) as singles,
            tc.tile_pool(name="scr", bufs=2) as scr,
            tc.tile_pool(name="ps", bufs=2, space="PSUM") as ps,
            tc.tile_pool(name="po", bufs=4, space="PSUM") as po,
        ):
            # ---- resident tiles ----
            xt_s = singles.tile([128, 2 * Li], F16, tag="xt_s")
            xb_s = singles.tile([128, NI * d], F16, tag="xb_s")
            mt_s = singles.tile([128, 2 * Lmp], F16, tag="mt_s")
            maug_s = singles.tile([128, NJ * D1], F16, tag="maug_s")
            mp_s = singles.tile([128, NJ], F32, tag="mp_s")
            dsc_s = singles.tile([128, 2], F32, tag="dsc_s")
            win_s = singles.tile([128, d], F16, tag="win_s")
            wmemc = singles.tile([128, 2], F16, tag="wmemc")
            ident16 = singles.tile([128, 128], F16, tag="ident16")

            PT = singles.tile([128, NJ * Li], F16, tag="PT")
            M1 = singles.tile([128, Li], F16, tag="M1")
            O1_all = singles.tile([128, NI * d], F16, tag="O1_all")
            B2_all = singles.tile([128, NI * d], F16, tag="B2_all")
            B3_all = singles.tile([128, NI * d], F16, tag="B3_all")
            xscr = singles.tile([128, NI * d], F16, tag="xscr")

            mvec = singles.tile([128, NJ], F32, tag="mvec")
            bias_sb = singles.tile([128, NJ], F32, tag="bias_sb")
            cmax = singles.tile([128, 1], F32, tag="cmax")
            cm_all = singles.tile([128, 1], F32, tag="cm_all")
            idot = singles.tile([128, NI], F32, tag="idot")
            maxP = singles.tile([128, NI], F32, tag="maxP")
            k1 = singles.tile([128, 1], F32, tag="k1")
            k_all = singles.tile([128, 1], F32, tag="k_all")
            negk = singles.tile([128, 1], F32, tag="negk")
            e2 = singles.tile([128, NI], F32, tag="e2")
            u_t = singles.tile([128, NI], F32, tag="u_t")
            su1 = singles.tile([128, 1], F32, tag="su1")
            su_all = singles.tile([128, 1], F32, tag="su_all")
            rec2 = singles.tile([128, 1], F32, tag="rec2")
            wt2 = singles.tile([128, NI], F16, tag="wt2")
            o2_1 = singles.tile([1, d], F32, tag="o2_1")
            o2b = singles.tile([128, d], F32, tag="o2b")
            o2b16 = singles.tile([128, d], F16, tag="o2b16")
            ones32 = singles.tile([128, 1], F32, tag="ones32")

            maug_r = maug_s[:].rearrange("p (c x) -> p c x", x=D1)

            # ==== loads: ONE queue in strict priority order ====
            # (all 8 cores load simultaneously; HBM contention splits bandwidth
            #  roughly evenly across ACTIVE queues — a single queue gives each
            #  critical transfer the full share, in order)
            # exactly 8 transfers on the sync ring — more would alias the 8
            # DMA-completion semaphore lanes and serialize behind reuse
            nc.sync.dma_start(out=ident16, in_=id_d[:, :])
            nc.sync.dma_start(out=dsc_s, in_=dsc_d[:, :])
            nc.sync.dma_start(out=wmemc, in_=wmemc_d[:, :])
            nc.sync.dma_start(out=mt_s, in_=mt_d[:, :])
            for kc in range(2):  # xt per kc-half; kc0 matmuls start first
                nc.sync.dma_start(out=xt_s[:, kc * Li:(kc + 1) * Li],
                                  in_=xt_d[:, kc * Li:(kc + 1) * Li])
            nc.sync.dma_start(out=maug_s, in_=maug_d[:, :])
            nc.sync.dma_start(out=xb_s, in_=xb_d[:, :])
            nc.gpsimd.dma_start(out=mp_s, in_=mp_d[:, :])
            nc.gpsimd.dma_start(out=win_s, in_=win_d[:, :])

            nc.vector.memset(ones32, 1.0)
            # load the ACT exp table during the load phase
            actwarm = scr.tile([1, 1], F32, tag="actw")
            nc.scalar.activation(out=actwarm, in_=ones32[0:1, 0:1], func=ACTF.Exp)

            # ==== PE warmup: un-throttle HAM while loads land ====
            for w in range(12):
                psw = po.tile([128, 128], F32, tag="po")
                nc.tensor.matmul(psw, ident16, ident16, start=True, stop=True)

            # ==== DVE preprocessing ====
            # fold dot_scale into x^T (per-partition scalar, in place)
            for kc in range(2):
                nc.vector.tensor_scalar_mul(
                    xt_s[:, kc * Li:(kc + 1) * Li],
                    xt_s[:, kc * Li:(kc + 1) * Li], dsc_s[:, kc:kc + 1])
            # mvec[j] = m[j,:] . w_mem  on the PE: 18 tiny matmuls against the
            # w_mem column, ready as soon as the mt chunks land (doubles as
            # real warmup work).  Full global max -> no fp16 exp overflow.
            psum_mv = po.tile([128, NJ], F32, tag="po")
            for jc in range(NJ):
                for kc in range(2):
                    nc.tensor.matmul(
                        psum_mv[:, jc:jc + 1],
                        mt_s[:, kc * Lmp + jc * 128: kc * Lmp + (jc + 1) * 128],
                        wmemc[:, kc:kc + 1],
                        start=(kc == 0), stop=(kc == 1))
            nc.vector.tensor_add(mvec, psum_mv, mp_s)
            nc.vector.reduce_max(out=cmax, in_=mvec, axis=AX.X)
            nc.gpsimd.partition_all_reduce(cm_all[:], cmax[:], channels=128,
                                           reduce_op=bass_isa.ReduceOp.max)
            nc.vector.tensor_scalar(
                out=bias_sb, in0=mvec, scalar1=cm_all[:, 0:1], scalar2=-4.0,
                op0=ALU.subtract, op1=ALU.add)

            # ==== phase 1 group: scores + exp + running max for (h, jc) ====
            def ph1_group(h, jc):
                psum_s = ps.tile([128, H], F32, tag="ps")
                for kc in range(2):
                    for bs in range(0, H, 512):  # fp16 moving operand max 512
                        nc.tensor.matmul(
                            psum_s[:, bs:bs + 512],
                            mt_s[:, kc * Lmp + jc * 128: kc * Lmp + (jc + 1) * 128],
                            xt_s[:, kc * Li + h * H + bs: kc * Li + h * H + bs + 512],
                            start=(kc == 0), stop=(kc == 1))
                pt_sl = PT[:, jc * Li + h * H: jc * Li + (h + 1) * H]
                nc.scalar.activation(out=pt_sl, in_=psum_s, func=ACTF.Exp,
                                     bias=bias_sb[:, jc:jc + 1], scale=1.0)
                m_sl = M1[:, h * H:(h + 1) * H]
                if jc == 0:
                    nc.vector.tensor_copy(m_sl, pt_sl)
                else:
                    nc.vector.tensor_max(m_sl, m_sl, pt_sl)

            # ==== phase 2 group + epilogue for i-tile it ====
            def ph2_group(it):
                psum_o = po.tile([128, D1], F32, tag="po")
                for jc in range(NJ):
                    nc.tensor.matmul(
                        psum_o,
                        PT[:, jc * Li + it * 128: jc * Li + (it + 1) * 128],
                        maug_r[:, jc, :],
                        start=(jc == 0), stop=(jc == NJ - 1))
                rec_s = scr.tile([128, 1], F32, tag="rec_s")
                nc.vector.reciprocal(rec_s, psum_o[:, d:d + 1])
                o1_sl = O1_all[:, it * d:(it + 1) * d]
                nc.scalar.activation(out=o1_sl, in_=psum_o[:, 0:d],
                                     func=ACTF.Copy, scale=rec_s[:, 0:1])

            def b2_batch(q):  # x*o1 for i-tiles [q*QT, (q+1)*QT)
                sl = slice(q * QT * d, (q + 1) * QT * d)
                nc.vector.tensor_mul(B2_all[:, sl], O1_all[:, sl], xb_s[:, sl])

            def store(src, col0, t0, nt=QT):
                nc.sync.dma_start(
                    out=out_d[t0 * 128:(t0 + nt) * 128,
                              col0:col0 + d].rearrange("(c p) x -> p c x", p=128),
                    in_=src[:, t0 * d:(t0 + nt) * d].rearrange(
                        "p (c x) -> p c x", x=d))

            # ---- phase 1 h=0 ----
            for jc in range(NJ):
                ph1_group(0, jc)

            # ---- phase 1 h=1 with phase-2 h=0 interleaved ----
            ph1_group(1, 0)
            ph1_group(1, 1)
            for jc in range(2, NJ):
                ph1_group(1, jc)
                ph2_group(jc - 2)          # it 0..6
            # idot[i] = x[i,:] . w_in  (chunked so no single op can block the
            # DVE queue; emitted late to keep its scheduler priority low)
            win_bc = win_s[:].unsqueeze(1).broadcast_to([128, QT, d])
            for q in range(NI // QT):
                sl = slice(q * QT * d, (q + 1) * QT * d)
                nc.vector.tensor_mul(
                    xscr[:, sl].rearrange("p (c x) -> p c x", x=d),
                    xb_s[:, sl].rearrange("p (c x) -> p c x", x=d), win_bc)
                nc.vector.reduce_sum(
                    out=idot[:, q * QT:(q + 1) * QT],
                    in_=xscr[:, sl].rearrange("p (c x) -> p c x", x=d),
                    axis=AX.X)
            # k chain for idot max (tiny, early)
            nc.vector.reduce_max(out=k1, in_=idot, axis=AX.X)
            nc.gpsimd.partition_all_reduce(k_all[:], k1[:], channels=128,
                                           reduce_op=bass_isa.ReduceOp.max)
            nc.vector.tensor_scalar_mul(negk, k_all, -1.0)
            ph2_group(7)
            b2_batch(0)
            b2_batch(1)
            store(O1_all, 0, 0)
            store(B2_all, d, 0)
            store(O1_all, 0, QT)
            store(B2_all, d, QT)

            # ---- stage C: maxP[i] = max over partitions of M1 (4-batched) ----
            for t0 in range(0, NI, 4):
                psT = po.tile([128, 512], F16, tag="po")
                for t in range(4):
                    nc.tensor.transpose(
                        psT[:, t * 128:(t + 1) * 128],
                        M1[:, (t0 + t) * 128:(t0 + t + 1) * 128], ident16)
                nc.vector.reduce_max(
                    out=maxP[:, t0:t0 + 4],
                    in_=psT[:].rearrange("p (c x) -> p c x", x=128), axis=AX.X)

            # ---- stage D: weight_two -> o2b ----
            ph2_group(8)
            ph2_group(9)
            nc.scalar.activation(out=e2, in_=idot, func=ACTF.Exp,
                                 bias=negk[:, 0:1], scale=1.0)
            nc.vector.tensor_mul(u_t, maxP, e2)
            nc.vector.reduce_sum(out=su1, in_=u_t, axis=AX.X)
            ph2_group(10)
            nc.gpsimd.partition_all_reduce(su_all[:], su1[:], channels=128,
                                           reduce_op=bass_isa.ReduceOp.add)
            nc.vector.reciprocal(rec2, su_all)
            nc.vector.tensor_scalar(out=wt2, in0=u_t, scalar1=rec2[:, 0:1],
                                    scalar2=None, op0=ALU.mult)
            ph2_group(11)
            psum_o2 = po.tile([1, d], F32, tag="po")
            for ic in range(NI):
                nc.tensor.matmul(psum_o2, wt2[:, ic:ic + 1],
                                 xb_s[:, ic * d:(ic + 1) * d],
                                 start=(ic == 0), stop=(ic == NI - 1))
            nc.vector.tensor_copy(o2_1, psum_o2)
            nc.gpsimd.partition_broadcast(o2b, o2_1)
            nc.vector.tensor_copy(o2b16, o2b)
            # block 3 for h=0 (o2b known; overlaps remaining phase 2).
            # NOTE: must stay off gpsimd — a gpsimd TensorTensor forces a Q7
            # library switch away from the all-reduce lib (~12us drain).
            o2_bc0 = o2b16[:].unsqueeze(1).broadcast_to([128, TPH, d])
            nc.vector.tensor_mul(
                B3_all[:, 0:TPH * d].rearrange("p (c x) -> p c x", x=d),
                O1_all[:, 0:TPH * d].rearrange("p (c x) -> p c x", x=d), o2_bc0)
            store(B3_all, 2 * d, 0)
            store(B3_all, 2 * d, QT)
            for it in range(12, NI):
                ph2_group(it)
            b2_batch(2)
            store(O1_all, 0, 2 * QT)
            store(B2_all, d, 2 * QT)
            o2_bc1 = o2b16[:].unsqueeze(1).broadcast_to([128, QT, d])
            nc.vector.tensor_mul(
                B3_all[:, 8 * d:12 * d].rearrange("p (c x) -> p c x", x=d),
                O1_all[:, 8 * d:12 * d].rearrange("p (c x) -> p c x", x=d),
                o2_bc1)
            store(B3_all, 2 * d, 2 * QT)
            # last quarter in two halves to shorten the store tail
            b2_batch(3)
            o2_bc2 = o2b16[:].unsqueeze(1).broadcast_to([128, 2, d])
            nc.vector.tensor_mul(
                B3_all[:, 12 * d:14 * d].rearrange("p (c x) -> p c x", x=d),
                O1_all[:, 12 * d:14 * d].rearrange("p (c x) -> p c x", x=d),
                o2_bc2)
            store(O1_all, 0, 12, 2)
            store(B2_all, d, 12, 2)
            store(B3_all, 2 * d, 12, 2)
            nc.vector.tensor_mul(
                B3_all[:, 14 * d:16 * d].rearrange("p (c x) -> p c x", x=d),
                O1_all[:, 14 * d:16 * d].rearrange("p (c x) -> p c x", x=d),
                o2_bc2)
            store(O1_all, 0, 14, 2)
            store(B2_all, d, 14, 2)
            store(B3_all, 2 * d, 14, 2)

    nc.compile()
    return nc


def _prep_core_inputs(x_b, m_b, mask_b, w_in, w_mem, dsc, Lmp):
    """Host-side shard prep: permute unmasked memory rows first, pad to Lmp,
    and marshal operands into the exact on-chip layouts (transpose / fp16 cast /
    constant padding only — all arithmetic happens on device)."""
    Li, d = x_b.shape
    NI, NJ, D1 = Li // 128, Lmp // 128, d + 1
    idx = np.flatnonzero(mask_b != 0)
    cnt = len(idx)
    m_p = np.zeros((Lmp, d), dtype=np.float32)
    m_p[:cnt] = m_b[idx]
    flat = np.zeros(Lmp, dtype=np.float32)
    flat[cnt:] = -NEG
    x16 = x_b.astype(np.float16)
    m16 = m_p.astype(np.float16)
    xt = np.ascontiguousarray(
        x16.T.reshape(2, 128, Li).transpose(1, 0, 2).reshape(128, 2 * Li))
    xb = np.ascontiguousarray(
        x16.reshape(NI, 128, d).transpose(1, 0, 2).reshape(128, NI * d))
    mt = np.ascontiguousarray(
        m16.T.reshape(2, 128, Lmp).transpose(1, 0, 2).reshape(128, 2 * Lmp))
    maug = np.ones((Lmp, D1), dtype=np.float16)
    maug[:, :d] = m16
    maug = np.ascontiguousarray(
        maug.reshape(NJ, 128, D1).transpose(1, 0, 2).reshape(128, NJ * D1))
    mp_t = np.ascontiguousarray(flat.reshape(NJ, 128).T)
    dsc_col = np.ascontiguousarray(np.asarray(dsc, np.float32).reshape(2, 128).T)
    winb = np.ascontiguousarray(
        np.broadcast_to(w_in.astype(np.float16)[None, :], (128, d)))
    wmemc = np.ascontiguousarray(
        w_mem.astype(np.float16).reshape(2, 128).T)
    id16 = np.eye(128, dtype=np.float16)
    return {"xt": xt, "xb": xb, "mt": mt, "maug": maug, "mp": mp_t,
            "dsc": dsc_col, "winb": winb, "wmemc": wmemc, "id16": id16}


def kernel(input, memory, mask, w_in, w_mem, dot_scale, _tmpdir=None):
    global LAST_RESULTS
    input = np.asarray(input, dtype=np.float32)
    memory = np.asarray(memory, dtype=np.float32)
    mask = np.asarray(mask)
    w_in = np.asarray(w_in, dtype=np.float32)
    w_mem = np.asarray(w_mem, dtype=np.float32)
    dot_scale = np.asarray(dot_scale, dtype=np.float32)

    bsz, Li, d = input.shape
    assert bsz == N_CORES

    counts = [int((mask[b] != 0).sum()) for b in range(bsz)]
    Lmp = max(128, int(math.ceil(max(counts) / 128.0)) * 128)

    key = (Li, Lmp, d)
    if key not in _NC_CACHE:
        _NC_CACHE[key] = build_nc(Li, Lmp, d)
    nc = _NC_CACHE[key]

    in_maps = [
        _prep_core_inputs(input[b], memory[b], mask[b], w_in, w_mem, dot_scale, Lmp)
        for b in range(bsz)
    ]
    res = run_bass_kernel_spmd(nc, in_maps, list(range(N_CORES)), tmpdir=_tmpdir)
    LAST_RESULTS = res
    out = np.empty((bsz, Li, 4 * d), dtype=np.float32)
    out[:, :, 0:d] = input
    for b in range(bsz):
        out[b, :, d:4 * d] = res.results[b]["out"].astype(np.float32)
    return out


# revision 45
# speedup vs baseline: 1.1134x; 1.1134x over previous
"""BiAttention Trainium2 Bass kernel (v9 — fp16 streaming pipeline).

Per-core (one batch per NeuronCore, batch=8 over 8 cores):
  att[i,j] = input_dot[i] + memory_dot[j] + (input*dot_scale) @ memory^T - NEG*(1-mask[j])
  weight_one = softmax_j(att);  output_one = weight_one @ memory
  weight_two = softmax_i(max_j att);  output_two = weight_two @ input
  out = concat([input, output_one, input*output_one, output_two*output_one], -1)

Implementation notes:
  - input_dot cancels in softmax_j; only memory_dot + mask pad enter the bias.
  - Unmasked memory rows are permuted first host-side; only Lmp rows computed.
  - Scores built transposed (S^T[j,i]): per-j bias is a per-partition ACT bias,
    exp lands in the P^T layout phase 2 needs.  C = max(mvec)+4 global shift.
  - Everything on-chip is fp16 (PT, operands, outputs); fp32 psum/stats.
  - dot_scale folded into x^T on device; mvec via tiny PE matmuls vs a w_mem
    column; idot via chunked DVE mul+reduce.
  - Cross-partition max/sum/broadcast via PE transpose + ones-row matmuls —
    gpsimd is left completely idle (its Q7 custom-op library load costs ~6us
    behind the load queue, and TensorTensor there forces a lib switch).
  - Loads: 5 packed fp16 transfers on one ring in priority order (smalls ride
    inside the first transfer; per-transfer fixed cost dominates small DMAs).
  - Device writes only the 3 computed output blocks in fp16; the `input` block
    is assembled host-side (pure copy).  Host prep is layout/dtype marshalling
    only — all arithmetic happens on device.
  - PE warmup matmuls on a memset tile un-throttle the HAM clock gate before
    any DMA lands.
"""

import math
import numpy as np

import concourse.bass as bass
import concourse.mybir as mybir
import concourse.tile as tile
import concourse.bacc as bacc
from concourse import bass_isa
from concourse.bass_utils import run_bass_kernel_spmd

F32 = mybir.dt.float32
F16 = mybir.dt.float16
AX = mybir.AxisListType
ALU = mybir.AluOpType
ACTF = mybir.ActivationFunctionType

N_CORES = 8
NEG = 1e30

_NC_CACHE: dict = {}
LAST_RESULTS = None  # BassKernelResults of the most recent run (for test harness)


def build_nc(Li: int, Lmp: int, d: int):
    """Build the single-core SPMD program.  Li, d fixed; Lmp = padded #unmasked."""
    assert Li % 128 == 0 and Lmp % 128 == 0 and d == 256
    NI = Li // 128          # i tiles (16)
    NJ = Lmp // 128         # j tiles (9)
    D1 = d + 1
    H = 1024                # phase-1 i-chunk
    NH = Li // H            # 2
    TPH = H // 128          # i tiles per chunk (8)
    QT = 4                  # i tiles per output store chunk

    # packed g1: ident16 | dsc(f16) | wmemc | mt
    G1W = 128 + 2 + 2 + 2 * Lmp
    # packed g4: xb | winb | mp(f16 +-inf)
    G4W = NI * d + d + NJ

    nc = bacc.Bacc("TRN2", target_bir_lowering=False, debug=False,
                   num_devices=N_CORES)

    g1_d = nc.dram_tensor("g1", [128, G1W], F16, kind="ExternalInput")
    xt_d = nc.dram_tensor("xt", [128, 2 * Li], F16, kind="ExternalInput")
    maug_d = nc.dram_tensor("maug", [128, NJ * D1], F16, kind="ExternalInput")
    g4_d = nc.dram_tensor("g4", [128, G4W], F16, kind="ExternalInput")
    out_d = nc.dram_tensor("out", [Li, 3 * d], F16, kind="ExternalOutput")

    with tile.TileContext(nc) as tc:
        with (
            tc.tile_pool(name="singles", bufs=1) as singles,
            tc.tile_pool(name="scr", bufs=2) as scr,
            tc.tile_pool(name="ps", bufs=2, space="PSUM") as ps,
            tc.tile_pool(name="po", bufs=4, space="PSUM") as po,
        ):
            # ---- resident tiles ----
            g1_s = singles.tile([128, G1W], F16, tag="g1_s")
            xt_s = singles.tile([128, 2 * Li], F16, tag="xt_s")
            maug_s = singles.tile([128, NJ * D1], F16, tag="maug_s")
            g4_s = singles.tile([128, G4W], F16, tag="g4_s")

            ident16 = g1_s[:, 0:128]
            dsc_s = g1_s[:, 128:130]
            wmemc = g1_s[:, 130:132]
            mt_s = g1_s[:, 132:132 + 2 * Lmp]
            xb_s = g4_s[:, 0:NI * d]
            win_s = g4_s[:, NI * d:NI * d + d]
            mp_s = g4_s[:, NI * d + d:NI * d + d + NJ]

            PT = singles.tile([128, NJ * Li], F16, tag="PT")
            M1 = singles.tile([128, Li], F16, tag="M1")
            O1_all = singles.tile([128, NI * d], F16, tag="O1_all")
            B2_all = singles.tile([128, NI * d], F16, tag="B2_all")
            B3_all = singles.tile([128, NI * d], F16, tag="B3_all")
            xscr = singles.tile([128, NI * d], F16, tag="xscr")

            mvec = singles.tile([128, NJ], F32, tag="mvec")
            bias_sb = singles.tile([128, NJ], F32, tag="bias_sb")
            cmax16 = singles.tile([128, 1], F16, tag="cmax16")
            cm1 = singles.tile([1, 1], F32, tag="cm1")
            cm_all = singles.tile([128, 1], F32, tag="cm_all")
            idot = singles.tile([128, NI], F32, tag="idot")
            maxP = singles.tile([128, NI], F32, tag="maxP")
            k116 = singles.tile([128, 1], F16, tag="k116")
            k11 = singles.tile([1, 1], F32, tag="k11")
            negk = singles.tile([128, 1], F32, tag="negk")
            e2 = singles.tile([128, NI], F32, tag="e2")
            u_t = singles.tile([128, NI], F32, tag="u_t")
            su1 = singles.tile([128, 1], F32, tag="su1")
            su_all = singles.tile([128, 1], F32, tag="su_all")
            rec2 = singles.tile([128, 1], F32, tag="rec2")
            wt2 = singles.tile([128, NI], F16, tag="wt2")
            o2b16 = singles.tile([128, d], F16, tag="o2b16")
            ones32 = singles.tile([128, 1], F32, tag="ones32")
            wz = singles.tile([128, 128], F16, tag="wz")

            maug_r = maug_s[:].rearrange("p (c x) -> p c x", x=D1)

            # ==== loads: 5 packed transfers, one ring, priority order ====
            nc.sync.dma_start(out=g1_s, in_=g1_d[:, :])
            for kc in range(2):  # xt per kc-half; kc0 matmuls start first
                nc.sync.dma_start(out=xt_s[:, kc * Li:(kc + 1) * Li],
                                  in_=xt_d[:, kc * Li:(kc + 1) * Li])
            nc.sync.dma_start(out=maug_s, in_=maug_d[:, :])
            nc.sync.dma_start(out=g4_s, in_=g4_d[:, :])

            nc.vector.memset(wz, 0.0)
            nc.vector.memset(ones32, 1.0)
            # load the ACT exp table during the load phase
            actwarm = scr.tile([1, 1], F32, tag="actw")
            nc.scalar.activation(out=actwarm, in_=ones32[0:1, 0:1], func=ACTF.Exp)

            # ==== PE warmup on the memset tile: no DMA dependency at all ====
            for w in range(16):
                psw = po.tile([128, 128], F32, tag="po")
                nc.tensor.matmul(psw, wz, wz, start=True, stop=True)

            # broadcast [1,1] -> [128,1] without gpsimd: replicate along the
            # free dim on DVE (stride-0 read), then transpose on the PE
            def bcast_col(dst_sb, src11):
                row = scr.tile([1, 128], F16, tag="brow")
                nc.vector.tensor_copy(row, src11[0:1, 0:1].broadcast_to([1, 128]))
                ps_b = po.tile([128, 1], F16, tag="po")
                nc.tensor.transpose(ps_b, row, ident16[0:1, 0:1])
                nc.vector.tensor_copy(dst_sb, ps_b)

            # ==== DVE preprocessing ====
            # fold dot_scale into x^T (per-partition scalar, in place)
            dsc32 = scr.tile([128, 2], F32, tag="dsc32")
            nc.vector.tensor_copy(dsc32, dsc_s)
            for kc in range(2):
                nc.vector.tensor_scalar_mul(
                    xt_s[:, kc * Li:(kc + 1) * Li],
                    xt_s[:, kc * Li:(kc + 1) * Li], dsc32[:, kc:kc + 1])
            # mvec[j] = m[j,:] . w_mem on the PE (tiny matmuls vs w_mem column)
            psum_mv = po.tile([128, NJ], F32, tag="po")
            for jc in range(NJ):
                for kc in range(2):
                    nc.tensor.matmul(
                        psum_mv[:, jc:jc + 1],
                        mt_s[:, kc * Lmp + jc * 128: kc * Lmp + (jc + 1) * 128],
                        wmemc[:, kc:kc + 1],
                        start=(kc == 0), stop=(kc == 1))
            nc.vector.tensor_add(mvec, psum_mv, mp_s)
            nc.vector.reduce_max(out=cmax16, in_=mvec, axis=AX.X)
            ps_c = po.tile([1, 128], F16, tag="po")
            nc.tensor.transpose(ps_c, cmax16, ident16)
            nc.vector.reduce_max(out=cm1, in_=ps_c, axis=AX.X)
            bcast_col(cm_all, cm1)
            nc.vector.tensor_scalar(
                out=bias_sb, in0=mvec, scalar1=cm_all[:, 0:1], scalar2=-4.0,
                op0=ALU.subtract, op1=ALU.add)

            # ==== phase 1 group: scores + exp + running max for (h, jc) ====
            def ph1_group(h, jc):
                psum_s = ps.tile([128, H], F32, tag="ps")
                for kc in range(2):
                    for bs in range(0, H, 512):  # fp16 moving operand max 512
                        nc.tensor.matmul(
                            psum_s[:, bs:bs + 512],
                            mt_s[:, kc * Lmp + jc * 128: kc * Lmp + (jc + 1) * 128],
                            xt_s[:, kc * Li + h * H + bs: kc * Li + h * H + bs + 512],
                            start=(kc == 0), stop=(kc == 1))
                pt_sl = PT[:, jc * Li + h * H: jc * Li + (h + 1) * H]
                nc.scalar.activation(out=pt_sl, in_=psum_s, func=ACTF.Exp,
                                     bias=bias_sb[:, jc:jc + 1], scale=1.0)
                m_sl = M1[:, h * H:(h + 1) * H]
                if jc == 0:
                    nc.vector.tensor_copy(m_sl, pt_sl)
                else:
                    nc.vector.tensor_max(m_sl, m_sl, pt_sl)

            # ==== phase 2 group + epilogue for i-tile it ====
            def ph2_group(it):
                psum_o = po.tile([128, D1], F32, tag="po")
                for jc in range(NJ):
                    nc.tensor.matmul(
                        psum_o,
                        PT[:, jc * Li + it * 128: jc * Li + (it + 1) * 128],
                        maug_r[:, jc, :],
                        start=(jc == 0), stop=(jc == NJ - 1))
                rec_s = scr.tile([128, 1], F32, tag="rec_s")
                nc.vector.reciprocal(rec_s, psum_o[:, d:d + 1])
                o1_sl = O1_all[:, it * d:(it + 1) * d]
                nc.scalar.activation(out=o1_sl, in_=psum_o[:, 0:d],
                                     func=ACTF.Copy, scale=rec_s[:, 0:1])

            def b2_batch(q):  # x*o1 for i-tiles [q*QT, (q+1)*QT)
                sl = slice(q * QT * d, (q + 1) * QT * d)
                nc.vector.tensor_mul(B2_all[:, sl], O1_all[:, sl], xb_s[:, sl])

            def store(src, col0, t0, nt=QT):
                nc.sync.dma_start(
                    out=out_d[t0 * 128:(t0 + nt) * 128,
                              col0:col0 + d].rearrange("(c p) x -> p c x", p=128),
                    in_=src[:, t0 * d:(t0 + nt) * d].rearrange(
                        "p (c x) -> p c x", x=d))

            # ---- phase 1 h=0 ----
            for jc in range(NJ):
                ph1_group(0, jc)

            # ---- phase 1 h=1 with phase-2 h=0 interleaved ----
            ph1_group(1, 0)
            ph1_group(1, 1)
            for jc in range(2, NJ):
                ph1_group(1, jc)
                ph2_group(jc - 2)          # it 0..6
            # idot[i] = x[i,:] . w_in  (chunked so no single op can block the
            # DVE queue; emitted late to keep its scheduler priority low)
            win_bc = win_s[:].unsqueeze(1).broadcast_to([128, QT, d])
            for q in range(NI // QT):
                sl = slice(q * QT * d, (q + 1) * QT * d)
                nc.vector.tensor_mul(
                    xscr[:, sl].rearrange("p (c x) -> p c x", x=d),
                    xb_s[:, sl].rearrange("p (c x) -> p c x", x=d), win_bc)
                nc.vector.reduce_sum(
                    out=idot[:, q * QT:(q + 1) * QT],
                    in_=xscr[:, sl].rearrange("p (c x) -> p c x", x=d),
                    axis=AX.X)
            # k chain for idot max (tiny, early)
            nc.vector.reduce_max(out=k116, in_=idot, axis=AX.X)
            ps_k = po.tile([1, 128], F16, tag="po")
            nc.tensor.transpose(ps_k, k116, ident16)
            nc.vector.reduce_max(out=k11, in_=ps_k, axis=AX.X)
            bcast_col(negk, k11)
            nc.vector.tensor_scalar_mul(negk, negk, -1.0)
            ph2_group(7)
            b2_batch(0)
            b2_batch(1)
            store(O1_all, 0, 0)
            store(B2_all, d, 0)
            store(O1_all, 0, QT)
            store(B2_all, d, QT)

            # ---- stage C: maxP[i] = max over partitions of M1 (4-batched) ----
            for t0 in range(0, NI, 4):
                psT = po.tile([128, 512], F16, tag="po")
                for t in range(4):
                    nc.tensor.transpose(
                        psT[:, t * 128:(t + 1) * 128],
                        M1[:, (t0 + t) * 128:(t0 + t + 1) * 128], ident16)
                nc.vector.reduce_max(
                    out=maxP[:, t0:t0 + 4],
                    in_=psT[:].rearrange("p (c x) -> p c x", x=128), axis=AX.X)

            # ---- stage D: weight_two -> o2b ----
            ph2_group(8)
            ph2_group(9)
            nc.scalar.activation(out=e2, in_=idot, func=ACTF.Exp,
                                 bias=negk[:, 0:1], scale=1.0)
            nc.vector.tensor_mul(u_t, maxP, e2)
            nc.vector.reduce_sum(out=su1, in_=u_t, axis=AX.X)
            ph2_group(10)
            # su_all[p] = sum_k su1[k] for every p: stride-0 stationary matmul
            ps_su = po.tile([128, 1], F32, tag="po")
            nc.tensor.matmul(ps_su, su1[:, 0:1].broadcast_to([128, 128]),
                             ones32, start=True, stop=True)
            nc.vector.tensor_copy(su_all, ps_su)
            nc.vector.reciprocal(rec2, su_all)
            nc.vector.tensor_scalar(out=wt2, in0=u_t, scalar1=rec2[:, 0:1],
                                    scalar2=None, op0=ALU.mult)
            ph2_group(11)
            # o2 replicated into all partitions directly: stationary is the
            # wt2 column broadcast along its free dim
            ps_o2b = po.tile([128, d], F32, tag="po")
            for ic in range(NI):
                nc.tensor.matmul(ps_o2b,
                                 wt2[:, ic:ic + 1].broadcast_to([128, 128]),
                                 xb_s[:, ic * d:(ic + 1) * d],
                                 start=(ic == 0), stop=(ic == NI - 1))
            nc.vector.tensor_copy(o2b16, ps_o2b)
            # block 3 for h=0 (o2b known; overlaps remaining phase 2)
            o2_bc0 = o2b16[:].unsqueeze(1).broadcast_to([128, TPH, d])
            nc.vector.tensor_mul(
                B3_all[:, 0:TPH * d].rearrange("p (c x) -> p c x", x=d),
                O1_all[:, 0:TPH * d].rearrange("p (c x) -> p c x", x=d), o2_bc0)
            store(B3_all, 2 * d, 0)
            store(B3_all, 2 * d, QT)
            for it in range(12, NI):
                ph2_group(it)
            b2_batch(2)
            store(O1_all, 0, 2 * QT)
            store(B2_all, d, 2 * QT)
            o2_bc1 = o2b16[:].unsqueeze(1).broadcast_to([128, QT, d])
            nc.vector.tensor_mul(
                B3_all[:, 8 * d:12 * d].rearrange("p (c x) -> p c x", x=d),
                O1_all[:, 8 * d:12 * d].rearrange("p (c x) -> p c x", x=d),
                o2_bc1)
            store(B3_all, 2 * d, 2 * QT)
            # last quarter in two halves to shorten the store tail
            b2_batch(3)
            o2_bc2 = o2b16[:].unsqueeze(1).broadcast_to([128, 2, d])
            nc.vector.tensor_mul(
                B3_all[:, 12 * d:14 * d].rearrange("p (c x) -> p c x", x=d),
                O1_all[:, 12 * d:14 * d].rearrange("p (c x) -> p c x", x=d),
                o2_bc2)
            store(O1_all, 0, 12, 2)
            store(B2_all, d, 12, 2)
            store(B3_all, 2 * d, 12, 2)
            nc.vector.tensor_mul(
                B3_all[:, 14 * d:16 * d].rearrange("p (c x) -> p c x", x=d),
                O1_all[:, 14 * d:16 * d].rearrange("p (c x) -> p c x", x=d),
                o2_bc2)
            store(O1_all, 0, 14, 2)
            store(B2_all, d, 14, 2)
            store(B3_all, 2 * d, 14, 2)

    nc.compile()
    return nc


def _prep_core_inputs(x_b, m_b, mask_b, w_in, w_mem, dsc, Lmp):
    """Host-side shard prep: permute unmasked memory rows first, pad to Lmp,
    and marshal operands into the exact on-chip layouts (transpose / fp16 cast /
    constant padding only — all arithmetic happens on device)."""
    Li, d = x_b.shape
    NI, NJ, D1 = Li // 128, Lmp // 128, d + 1
    idx = np.flatnonzero(mask_b != 0)
    cnt = len(idx)
    m_p = np.zeros((Lmp, d), dtype=np.float32)
    m_p[:cnt] = m_b[idx]
    x16 = x_b.astype(np.float16)
    m16 = m_p.astype(np.float16)
    xt = np.ascontiguousarray(
        x16.T.reshape(2, 128, Li).transpose(1, 0, 2).reshape(128, 2 * Li))
    mt = m16.T.reshape(2, 128, Lmp).transpose(1, 0, 2).reshape(128, 2 * Lmp)
    maug = np.ones((Lmp, D1), dtype=np.float16)
    maug[:, :d] = m16
    maug = np.ascontiguousarray(
        maug.reshape(NJ, 128, D1).transpose(1, 0, 2).reshape(128, NJ * D1))
    # g1: ident | dsc(f16) | wmemc | mt
    g1 = np.empty((128, 128 + 2 + 2 + 2 * Lmp), dtype=np.float16)
    g1[:, 0:128] = np.eye(128, dtype=np.float16)
    g1[:, 128:130] = dsc.astype(np.float16).reshape(2, 128).T
    g1[:, 130:132] = w_mem.astype(np.float16).reshape(2, 128).T
    g1[:, 132:] = mt
    # g4: xb | winb | mp (as f16; -1e30 -> -inf is fine for the bias path)
    flat = np.zeros(Lmp, dtype=np.float16)
    flat[cnt:] = np.float16(-65504.0)  # f16 lowest; exp underflows to 0
    g4 = np.empty((128, NI * d + d + NJ), dtype=np.float16)
    g4[:, 0:NI * d] = x16.reshape(NI, 128, d).transpose(1, 0, 2).reshape(
        128, NI * d)
    g4[:, NI * d:NI * d + d] = np.broadcast_to(
        w_in.astype(np.float16)[None, :], (128, d))
    g4[:, NI * d + d:] = flat.reshape(NJ, 128).T
    return {"g1": np.ascontiguousarray(g1), "xt": xt,
            "maug": maug, "g4": np.ascontiguousarray(g4)}


def kernel(input, memory, mask, w_in, w_mem, dot_scale, _tmpdir=None):
    global LAST_RESULTS
    input = np.asarray(input, dtype=np.float32)
    memory = np.asarray(memory, dtype=np.float32)
    mask = np.asarray(mask)
    w_in = np.asarray(w_in, dtype=np.float32)
    w_mem = np.asarray(w_mem, dtype=np.float32)
    dot_scale = np.asarray(dot_scale, dtype=np.float32)

    bsz, Li, d = input.shape
    assert bsz == N_CORES

    counts = [int((mask[b] != 0).sum()) for b in range(bsz)]
    Lmp = max(128, int(math.ceil(max(counts) / 128.0)) * 128)

    key = (Li, Lmp, d)
    if key not in _NC_CACHE:
        _NC_CACHE[key] = build_nc(Li, Lmp, d)
    nc = _NC_CACHE[key]

    in_maps = [
        _prep_core_inputs(input[b], memory[b], mask[b], w_in, w_mem, dot_scale, Lmp)
        for b in range(bsz)
    ]
    res = run_bass_kernel_spmd(nc, in_maps, list(range(N_CORES)), tmpdir=_tmpdir)
    LAST_RESULTS = res
    out = np.empty((bsz, Li, 4 * d), dtype=np.float32)
    out[:, :, 0:d] = input
    for b in range(bsz):
        out[b, :, d:4 * d] = res.results[b]["out"].astype(np.float32)
    return out


# revision 47
# speedup vs baseline: 1.1517x; 1.0343x over previous
"""BiAttention Trainium2 Bass kernel (v9 — fp16 streaming pipeline).

Per-core (one batch per NeuronCore, batch=8 over 8 cores):
  att[i,j] = input_dot[i] + memory_dot[j] + (input*dot_scale) @ memory^T - NEG*(1-mask[j])
  weight_one = softmax_j(att);  output_one = weight_one @ memory
  weight_two = softmax_i(max_j att);  output_two = weight_two @ input
  out = concat([input, output_one, input*output_one, output_two*output_one], -1)

Implementation notes:
  - input_dot cancels in softmax_j; only memory_dot + mask pad enter the bias.
  - Unmasked memory rows are permuted first host-side; only Lmp rows computed.
  - Scores built transposed (S^T[j,i]): per-j bias is a per-partition ACT bias,
    exp lands in the P^T layout phase 2 needs.  C = max(mvec)+4 global shift.
  - Everything on-chip is fp16 (PT, operands, outputs); fp32 psum/stats.
  - dot_scale folded into x^T on device; mvec via tiny PE matmuls vs a w_mem
    column; idot via chunked DVE mul+reduce.
  - Cross-partition max/sum/broadcast via PE transpose + ones-row matmuls —
    gpsimd is left completely idle (its Q7 custom-op library load costs ~6us
    behind the load queue, and TensorTensor there forces a lib switch).
  - Loads: 5 packed fp16 transfers on one ring in priority order (smalls ride
    inside the first transfer; per-transfer fixed cost dominates small DMAs).
  - Device writes only the 3 computed output blocks in fp16; the `input` block
    is assembled host-side (pure copy).  Host prep is layout/dtype marshalling
    only — all arithmetic happens on device.
  - PE warmup matmuls on a memset tile un-throttle the HAM clock gate before
    any DMA lands.
"""

import math
import numpy as np

import concourse.bass as bass
import concourse.mybir as mybir
import concourse.tile as tile
import concourse.bacc as bacc
from concourse import bass_isa
from concourse.bass_utils import run_bass_kernel_spmd

F32 = mybir.dt.float32
F16 = mybir.dt.float16
AX = mybir.AxisListType
ALU = mybir.AluOpType
ACTF = mybir.ActivationFunctionType

N_CORES = 8
NEG = 1e30

_NC_CACHE: dict = {}
LAST_RESULTS = None  # BassKernelResults of the most recent run (for test harness)


def build_nc(Li: int, Lmp: int, d: int):
    """Build the single-core SPMD program.  Li, d fixed; Lmp = padded #unmasked."""
    assert Li % 128 == 0 and Lmp % 128 == 0 and d == 256
    NI = Li // 128          # i tiles (16)
    NJ = Lmp // 128         # j tiles (9)
    D1 = d + 1
    H = 1024                # phase-1 i-chunk
    NH = Li // H            # 2
    TPH = H // 128          # i tiles per chunk (8)
    QT = 4                  # i tiles per output store chunk

    # packed g1: ident16 | dsc(f16) | wmemc | winc | mt
    G1W = 128 + 2 + 2 + 2 + 2 * Lmp
    # packed g4: xb | mp(f16)
    G4W = NI * d + NJ

    nc = bacc.Bacc("TRN2", target_bir_lowering=False, debug=False,
                   num_devices=N_CORES)

    g1_d = nc.dram_tensor("g1", [128, G1W], F16, kind="ExternalInput")
    xt_d = nc.dram_tensor("xt", [128, 2 * Li], F16, kind="ExternalInput")
    maug_d = nc.dram_tensor("maug", [128, NJ * D1], F16, kind="ExternalInput")
    g4_d = nc.dram_tensor("g4", [128, G4W], F16, kind="ExternalInput")
    out_d = nc.dram_tensor("out", [Li, 3 * d], F16, kind="ExternalOutput")

    with tile.TileContext(nc) as tc:
        with (
            tc.tile_pool(name="singles", bufs=1) as singles,
            tc.tile_pool(name="scr", bufs=2) as scr,
            tc.tile_pool(name="ps", bufs=2, space="PSUM") as ps,
            tc.tile_pool(name="po", bufs=4, space="PSUM") as po,
        ):
            # ---- resident tiles ----
            g1_s = singles.tile([128, G1W], F16, tag="g1_s")
            xt_s = singles.tile([128, 2 * Li], F16, tag="xt_s")
            xts = singles.tile([128, 2 * Li], F16, tag="xts")
            maug_s = singles.tile([128, NJ * D1], F16, tag="maug_s")
            g4_s = singles.tile([128, G4W], F16, tag="g4_s")

            ident16 = g1_s[:, 0:128]
            dsc_s = g1_s[:, 128:130]
            wmemc = g1_s[:, 130:132]
            winc = g1_s[:, 132:134]
            mt_s = g1_s[:, 134:134 + 2 * Lmp]
            xb_s = g4_s[:, 0:NI * d]
            mp_s = g4_s[:, NI * d:NI * d + NJ]

            PT = singles.tile([128, NJ * Li], F16, tag="PT")
            M1 = singles.tile([128, Li], F16, tag="M1")
            O1_all = singles.tile([128, NI * d], F16, tag="O1_all")
            B2_all = singles.tile([128, NI * d], F16, tag="B2_all")
            B3_all = singles.tile([128, NI * d], F16, tag="B3_all")

            mvec = singles.tile([128, NJ], F32, tag="mvec")
            bias_sb = singles.tile([128, NJ], F32, tag="bias_sb")
            cmax16 = singles.tile([128, 1], F16, tag="cmax16")
            cm1 = singles.tile([1, 1], F32, tag="cm1")
            cm_all = singles.tile([128, 1], F32, tag="cm_all")
            idot = singles.tile([128, NI], F32, tag="idot")
            maxP = singles.tile([128, NI], F32, tag="maxP")
            k116 = singles.tile([128, 1], F16, tag="k116")
            k11 = singles.tile([1, 1], F32, tag="k11")
            negk = singles.tile([128, 1], F32, tag="negk")
            e2 = singles.tile([128, NI], F32, tag="e2")
            u_t = singles.tile([128, NI], F32, tag="u_t")
            su1 = singles.tile([128, 1], F32, tag="su1")
            su_all = singles.tile([128, 1], F32, tag="su_all")
            rec2 = singles.tile([128, 1], F32, tag="rec2")
            wt2 = singles.tile([128, NI], F16, tag="wt2")
            o2b16 = singles.tile([128, d], F16, tag="o2b16")
            ones32 = singles.tile([128, 1], F32, tag="ones32")
            wz = singles.tile([128, 128], F16, tag="wz")

            maug_r = maug_s[:].rearrange("p (c x) -> p c x", x=D1)

            # ==== loads: 5 packed transfers, one ring, priority order ====
            nc.sync.dma_start(out=g1_s, in_=g1_d[:, :])
            for kc in range(2):  # xt per kc-half; kc0 matmuls start first
                nc.sync.dma_start(out=xt_s[:, kc * Li:(kc + 1) * Li],
                                  in_=xt_d[:, kc * Li:(kc + 1) * Li])
            nc.sync.dma_start(out=maug_s, in_=maug_d[:, :])
            nc.sync.dma_start(out=g4_s, in_=g4_d[:, :])

            nc.vector.memset(wz, 0.0)
            nc.vector.memset(ones32, 1.0)
            # load the ACT exp table during the load phase
            actwarm = scr.tile([1, 1], F32, tag="actw")
            nc.scalar.activation(out=actwarm, in_=ones32[0:1, 0:1], func=ACTF.Exp)

            # ==== PE warmup on the memset tile: no DMA dependency at all ====
            for w in range(16):
                psw = po.tile([128, 128], F32, tag="po")
                nc.tensor.matmul(psw, wz, wz, start=True, stop=True)

            # broadcast [1,1] -> [128,1] without gpsimd: replicate along the
            # free dim on DVE (stride-0 read), then transpose on the PE
            def bcast_col(dst_sb, src11):
                row = scr.tile([1, 128], F16, tag="brow")
                nc.vector.tensor_copy(row, src11[0:1, 0:1].broadcast_to([1, 128]))
                ps_b = po.tile([128, 1], F16, tag="po")
                nc.tensor.transpose(ps_b, row, ident16[0:1, 0:1])
                nc.vector.tensor_copy(dst_sb, ps_b)

            # ==== DVE preprocessing ====
            # fold dot_scale into x^T (per-partition scalar, in place)
            dsc32 = scr.tile([128, 2], F32, tag="dsc32")
            nc.vector.tensor_copy(dsc32, dsc_s)
            for kc in range(2):
                nc.vector.tensor_scalar_mul(
                    xts[:, kc * Li:(kc + 1) * Li],
                    xt_s[:, kc * Li:(kc + 1) * Li], dsc32[:, kc:kc + 1])
            # mvec[j] = m[j,:] . w_mem on the PE (tiny matmuls vs w_mem column)
            psum_mv = po.tile([128, NJ], F32, tag="po")
            for jc in range(NJ):
                for kc in range(2):
                    nc.tensor.matmul(
                        psum_mv[:, jc:jc + 1],
                        mt_s[:, kc * Lmp + jc * 128: kc * Lmp + (jc + 1) * 128],
                        wmemc[:, kc:kc + 1],
                        start=(kc == 0), stop=(kc == 1))
            nc.vector.tensor_add(mvec, psum_mv, mp_s)
            nc.vector.reduce_max(out=cmax16, in_=mvec, axis=AX.X)
            ps_c = po.tile([1, 128], F16, tag="po")
            nc.tensor.transpose(ps_c, cmax16, ident16)
            nc.vector.reduce_max(out=cm1, in_=ps_c, axis=AX.X)
            bcast_col(cm_all, cm1)
            nc.vector.tensor_scalar(
                out=bias_sb, in0=mvec, scalar1=cm_all[:, 0:1], scalar2=-4.0,
                op0=ALU.subtract, op1=ALU.add)

            # ==== phase 1 group: scores + exp + running max for (h, jc) ====
            def ph1_group(h, jc):
                psum_s = ps.tile([128, H], F32, tag="ps")
                for kc in range(2):
                    for bs in range(0, H, 512):  # fp16 moving operand max 512
                        nc.tensor.matmul(
                            psum_s[:, bs:bs + 512],
                            mt_s[:, kc * Lmp + jc * 128: kc * Lmp + (jc + 1) * 128],
                            xts[:, kc * Li + h * H + bs: kc * Li + h * H + bs + 512],
                            start=(kc == 0), stop=(kc == 1))
                pt_sl = PT[:, jc * Li + h * H: jc * Li + (h + 1) * H]
                nc.scalar.activation(out=pt_sl, in_=psum_s, func=ACTF.Exp,
                                     bias=bias_sb[:, jc:jc + 1], scale=1.0)
                m_sl = M1[:, h * H:(h + 1) * H]
                if jc == 0:
                    nc.vector.tensor_copy(m_sl, pt_sl)
                else:
                    nc.vector.tensor_max(m_sl, m_sl, pt_sl)

            # ==== phase 2 group + epilogue for i-tile it ====
            def ph2_group(it):
                psum_o = po.tile([128, D1], F32, tag="po")
                for jc in range(NJ):
                    nc.tensor.matmul(
                        psum_o,
                        PT[:, jc * Li + it * 128: jc * Li + (it + 1) * 128],
                        maug_r[:, jc, :],
                        start=(jc == 0), stop=(jc == NJ - 1))
                rec_s = scr.tile([128, 1], F32, tag="rec_s")
                nc.vector.reciprocal(rec_s, psum_o[:, d:d + 1])
                o1_sl = O1_all[:, it * d:(it + 1) * d]
                nc.scalar.activation(out=o1_sl, in_=psum_o[:, 0:d],
                                     func=ACTF.Copy, scale=rec_s[:, 0:1])

            def b2_batch(q):  # x*o1 for i-tiles [q*QT, (q+1)*QT)
                sl = slice(q * QT * d, (q + 1) * QT * d)
                nc.vector.tensor_mul(B2_all[:, sl], O1_all[:, sl], xb_s[:, sl])

            def store(src, col0, t0, nt=QT):
                nc.sync.dma_start(
                    out=out_d[t0 * 128:(t0 + nt) * 128,
                              col0:col0 + d].rearrange("(c p) x -> p c x", p=128),
                    in_=src[:, t0 * d:(t0 + nt) * d].rearrange(
                        "p (c x) -> p c x", x=d))

            # ---- phase 1 h=0 ----
            for jc in range(NJ):
                ph1_group(0, jc)

            # ---- phase 1 h=1 with phase-2 h=0 interleaved ----
            ph1_group(1, 0)
            ph1_group(1, 1)
            for jc in range(2, NJ):
                ph1_group(1, jc)
                ph2_group(jc - 2)          # it 0..6
            # idot[i] = x[i,:] . w_in on the PE: raw x^T tiles against the
            # w_in column (fills PE stalls during the exp stream; keeps the
            # DVE queue free of load-gated work)
            psum_id = po.tile([128, NI], F32, tag="po")
            for it in range(NI):
                for kc in range(2):
                    nc.tensor.matmul(
                        psum_id[:, it:it + 1],
                        xt_s[:, kc * Li + it * 128: kc * Li + (it + 1) * 128],
                        winc[:, kc:kc + 1],
                        start=(kc == 0), stop=(kc == 1))
            nc.vector.tensor_copy(idot, psum_id)
            # k chain for idot max (tiny, early)
            nc.vector.reduce_max(out=k116, in_=idot, axis=AX.X)
            ps_k = po.tile([1, 128], F16, tag="po")
            nc.tensor.transpose(ps_k, k116, ident16)
            nc.vector.reduce_max(out=k11, in_=ps_k, axis=AX.X)
            bcast_col(negk, k11)
            nc.vector.tensor_scalar_mul(negk, negk, -1.0)
            ph2_group(7)
            b2_batch(0)
            b2_batch(1)
            store(O1_all, 0, 0)
            store(B2_all, d, 0)
            store(O1_all, 0, QT)
            store(B2_all, d, QT)

            # ---- stage C: maxP[i] = max over partitions of M1 (4-batched) ----
            for t0 in range(0, NI, 4):
                psT = po.tile([128, 512], F16, tag="po")
                for t in range(4):
                    nc.tensor.transpose(
                        psT[:, t * 128:(t + 1) * 128],
                        M1[:, (t0 + t) * 128:(t0 + t + 1) * 128], ident16)
                nc.vector.reduce_max(
                    out=maxP[:, t0:t0 + 4],
                    in_=psT[:].rearrange("p (c x) -> p c x", x=128), axis=AX.X)

            # ---- stage D: weight_two -> o2b ----
            ph2_group(8)
            ph2_group(9)
            nc.scalar.activation(out=e2, in_=idot, func=ACTF.Exp,
                                 bias=negk[:, 0:1], scale=1.0)
            nc.vector.tensor_mul(u_t, maxP, e2)
            nc.vector.reduce_sum(out=su1, in_=u_t, axis=AX.X)
            ph2_group(10)
            # su_all[p] = sum_k su1[k] for every p: stride-0 stationary matmul
            ps_su = po.tile([128, 1], F32, tag="po")
            nc.tensor.matmul(ps_su, su1[:, 0:1].broadcast_to([128, 128]),
                             ones32, start=True, stop=True)
            nc.vector.tensor_copy(su_all, ps_su)
            nc.vector.reciprocal(rec2, su_all)
            nc.vector.tensor_scalar(out=wt2, in0=u_t, scalar1=rec2[:, 0:1],
                                    scalar2=None, op0=ALU.mult)
            ph2_group(11)
            # o2 replicated into all partitions directly: stationary is the
            # wt2 column broadcast along its free dim
            ps_o2b = po.tile([128, d], F32, tag="po")
            for ic in range(NI):
                nc.tensor.matmul(ps_o2b,
                                 wt2[:, ic:ic + 1].broadcast_to([128, 128]),
                                 xb_s[:, ic * d:(ic + 1) * d],
                                 start=(ic == 0), stop=(ic == NI - 1))
            nc.vector.tensor_copy(o2b16, ps_o2b)
            # block 3 for h=0 (o2b known; overlaps remaining phase 2)
            o2_bc0 = o2b16[:].unsqueeze(1).broadcast_to([128, TPH, d])
            nc.vector.tensor_mul(
                B3_all[:, 0:TPH * d].rearrange("p (c x) -> p c x", x=d),
                O1_all[:, 0:TPH * d].rearrange("p (c x) -> p c x", x=d), o2_bc0)
            store(B3_all, 2 * d, 0)
            store(B3_all, 2 * d, QT)
            for it in range(12, NI):
                ph2_group(it)
            b2_batch(2)
            store(O1_all, 0, 2 * QT)
            store(B2_all, d, 2 * QT)
            o2_bc1 = o2b16[:].unsqueeze(1).broadcast_to([128, QT, d])
            nc.vector.tensor_mul(
                B3_all[:, 8 * d:12 * d].rearrange("p (c x) -> p c x", x=d),
                O1_all[:, 8 * d:12 * d].rearrange("p (c x) -> p c x", x=d),
                o2_bc1)
            store(B3_all, 2 * d, 2 * QT)
            # last quarter in two halves to shorten the store tail
            b2_batch(3)
            o2_bc2 = o2b16[:].unsqueeze(1).broadcast_to([128, 2, d])
            nc.vector.tensor_mul(
                B3_all[:, 12 * d:14 * d].rearrange("p (c x) -> p c x", x=d),
                O1_all[:, 12 * d:14 * d].rearrange("p (c x) -> p c x", x=d),
                o2_bc2)
            store(O1_all, 0, 12, 2)
            store(B2_all, d, 12, 2)
            store(B3_all, 2 * d, 12, 2)
            nc.vector.tensor_mul(
                B3_all[:, 14 * d:16 * d].rearrange("p (c x) -> p c x", x=d),
                O1_all[:, 14 * d:16 * d].rearrange("p (c x) -> p c x", x=d),
                o2_bc2)
            store(O1_all, 0, 14, 2)
            store(B2_all, d, 14, 2)
            store(B3_all, 2 * d, 14, 2)

    nc.compile()
    return nc


def _prep_core_inputs(x_b, m_b, mask_b, w_in, w_mem, dsc, Lmp):
    """Host-side shard prep: permute unmasked memory rows first, pad to Lmp,
    and marshal operands into the exact on-chip layouts (transpose / fp16 cast /
    constant padding only — all arithmetic happens on device)."""
    Li, d = x_b.shape
    NI, NJ, D1 = Li // 128, Lmp // 128, d + 1
    idx = np.flatnonzero(mask_b != 0)
    cnt = len(idx)
    m_p = np.zeros((Lmp, d), dtype=np.float32)
    m_p[:cnt] = m_b[idx]
    x16 = x_b.astype(np.float16)
    m16 = m_p.astype(np.float16)
    xt = np.ascontiguousarray(
        x16.T.reshape(2, 128, Li).transpose(1, 0, 2).reshape(128, 2 * Li))
    mt = m16.T.reshape(2, 128, Lmp).transpose(1, 0, 2).reshape(128, 2 * Lmp)
    maug = np.ones((Lmp, D1), dtype=np.float16)
    maug[:, :d] = m16
    maug = np.ascontiguousarray(
        maug.reshape(NJ, 128, D1).transpose(1, 0, 2).reshape(128, NJ * D1))
    # g1: ident | dsc(f16) | wmemc | winc | mt
    g1 = np.empty((128, 128 + 2 + 2 + 2 + 2 * Lmp), dtype=np.float16)
    g1[:, 0:128] = np.eye(128, dtype=np.float16)
    g1[:, 128:130] = dsc.astype(np.float16).reshape(2, 128).T
    g1[:, 130:132] = w_mem.astype(np.float16).reshape(2, 128).T
    g1[:, 132:134] = w_in.astype(np.float16).reshape(2, 128).T
    g1[:, 134:] = mt
    # g4: xb | mp (as f16; exp of the f16-lowest pad underflows to 0)
    flat = np.zeros(Lmp, dtype=np.float16)
    flat[cnt:] = np.float16(-65504.0)
    g4 = np.empty((128, NI * d + NJ), dtype=np.float16)
    g4[:, 0:NI * d] = x16.reshape(NI, 128, d).transpose(1, 0, 2).reshape(
        128, NI * d)
    g4[:, NI * d:] = flat.reshape(NJ, 128).T
    return {"g1": np.ascontiguousarray(g1), "xt": xt,
            "maug": maug, "g4": np.ascontiguousarray(g4)}


def kernel(input, memory, mask, w_in, w_mem, dot_scale, _tmpdir=None):
    global LAST_RESULTS
    input = np.asarray(input, dtype=np.float32)
    memory = np.asarray(memory, dtype=np.float32)
    mask = np.asarray(mask)
    w_in = np.asarray(w_in, dtype=np.float32)
    w_mem = np.asarray(w_mem, dtype=np.float32)
    dot_scale = np.asarray(dot_scale, dtype=np.float32)

    bsz, Li, d = input.shape
    assert bsz == N_CORES

    counts = [int((mask[b] != 0).sum()) for b in range(bsz)]
    Lmp = max(128, int(math.ceil(max(counts) / 128.0)) * 128)

    key = (Li, Lmp, d)
    if key not in _NC_CACHE:
        _NC_CACHE[key] = build_nc(Li, Lmp, d)
    nc = _NC_CACHE[key]

    in_maps = [
        _prep_core_inputs(input[b], memory[b], mask[b], w_in, w_mem, dot_scale, Lmp)
        for b in range(bsz)
    ]
    res = run_bass_kernel_spmd(nc, in_maps, list(range(N_CORES)), tmpdir=_tmpdir)
    LAST_RESULTS = res
    out = np.empty((bsz, Li, 4 * d), dtype=np.float32)
    out[:, :, 0:d] = input
    for b in range(bsz):
        out[b, :, d:4 * d] = res.results[b]["out"].astype(np.float32)
    return out


# revision 48
# speedup vs baseline: 1.2404x; 1.0770x over previous
"""BiAttention Trainium2 Bass kernel (v9 — fp16 streaming pipeline).

Per-core (one batch per NeuronCore, batch=8 over 8 cores):
  att[i,j] = input_dot[i] + memory_dot[j] + (input*dot_scale) @ memory^T - NEG*(1-mask[j])
  weight_one = softmax_j(att);  output_one = weight_one @ memory
  weight_two = softmax_i(max_j att);  output_two = weight_two @ input
  out = concat([input, output_one, input*output_one, output_two*output_one], -1)

Implementation notes:
  - input_dot cancels in softmax_j; only memory_dot + mask pad enter the bias.
  - Unmasked memory rows are permuted first host-side; only Lmp rows computed.
  - Scores built transposed (S^T[j,i]): per-j bias is a per-partition ACT bias,
    exp lands in the P^T layout phase 2 needs.  C = max(mvec)+4 global shift.
  - Everything on-chip is fp16 (PT, operands, outputs); fp32 psum/stats.
  - dot_scale folded into x^T on device; mvec via tiny PE matmuls vs a w_mem
    column; idot via chunked DVE mul+reduce.
  - Cross-partition max/sum/broadcast via PE transpose + ones-row matmuls —
    gpsimd is left completely idle (its Q7 custom-op library load costs ~6us
    behind the load queue, and TensorTensor there forces a lib switch).
  - Loads: 5 packed fp16 transfers on one ring in priority order (smalls ride
    inside the first transfer; per-transfer fixed cost dominates small DMAs).
  - Device writes only the 3 computed output blocks in fp16; the `input` block
    is assembled host-side (pure copy).  Host prep is layout/dtype marshalling
    only — all arithmetic happens on device.
  - PE warmup matmuls on a memset tile un-throttle the HAM clock gate before
    any DMA lands.
"""

import math
import numpy as np

import concourse.bass as bass
import concourse.mybir as mybir
import concourse.tile as tile
import concourse.bacc as bacc
from concourse import bass_isa
from concourse.bass_utils import run_bass_kernel_spmd

F32 = mybir.dt.float32
F16 = mybir.dt.float16
AX = mybir.AxisListType
ALU = mybir.AluOpType
ACTF = mybir.ActivationFunctionType

N_CORES = 8
NEG = 1e30

_NC_CACHE: dict = {}
LAST_RESULTS = None  # BassKernelResults of the most recent run (for test harness)


def build_nc(Li: int, Lmp: int, d: int):
    """Build the single-core SPMD program.  Li, d fixed; Lmp = padded #unmasked."""
    assert Li % 128 == 0 and Lmp % 128 == 0 and d == 256
    NI = Li // 128          # i tiles (16)
    NJ = Lmp // 128         # j tiles (9)
    D1 = d + 1
    H = 1024                # phase-1 i-chunk
    NH = Li // H            # 2
    TPH = H // 128          # i tiles per chunk (8)
    QT = 4                  # i tiles per output store chunk

    # packed g1: ident16 | dsc(f16) | wmemc | winc | mp | mt
    G1W = 128 + 2 + 2 + 2 + NJ + 2 * Lmp
    # packed g4: xb
    G4W = NI * d

    nc = bacc.Bacc("TRN2", target_bir_lowering=False, debug=False,
                   num_devices=N_CORES)

    g1_d = nc.dram_tensor("g1", [128, G1W], F16, kind="ExternalInput")
    xt_d = nc.dram_tensor("xt", [128, 2 * Li], F16, kind="ExternalInput")
    maug_d = nc.dram_tensor("maug", [128, NJ * D1], F16, kind="ExternalInput")
    g4_d = nc.dram_tensor("g4", [128, G4W], F16, kind="ExternalInput")
    out_d = nc.dram_tensor("out", [Li, 3 * d], F16, kind="ExternalOutput")

    with tile.TileContext(nc) as tc:
        with (
            tc.tile_pool(name="singles", bufs=1) as singles,
            tc.tile_pool(name="scr", bufs=2) as scr,
            tc.tile_pool(name="ps", bufs=2, space="PSUM") as ps,
            tc.tile_pool(name="po", bufs=4, space="PSUM") as po,
        ):
            # ---- resident tiles ----
            g1_s = singles.tile([128, G1W], F16, tag="g1_s")
            xt_s = singles.tile([128, 2 * Li], F16, tag="xt_s")
            xts = singles.tile([128, 2 * Li], F16, tag="xts")
            maug_s = singles.tile([128, NJ * D1], F16, tag="maug_s")
            g4_s = singles.tile([128, G4W], F16, tag="g4_s")

            ident16 = g1_s[:, 0:128]
            dsc_s = g1_s[:, 128:130]
            wmemc = g1_s[:, 130:132]
            winc = g1_s[:, 132:134]
            mp_s = g1_s[:, 134:134 + NJ]
            mt_s = g1_s[:, 134 + NJ:134 + NJ + 2 * Lmp]
            xb_s = g4_s[:, 0:NI * d]

            PT = singles.tile([128, NJ * Li], F16, tag="PT")
            M1 = singles.tile([128, Li], F16, tag="M1")
            O1_all = singles.tile([128, NI * d], F16, tag="O1_all")
            B2_all = singles.tile([128, NI * d], F16, tag="B2_all")
            B3_all = singles.tile([128, NI * d], F16, tag="B3_all")

            mvec = singles.tile([128, NJ], F32, tag="mvec")
            bias_sb = singles.tile([128, NJ], F32, tag="bias_sb")
            cmax16 = singles.tile([128, 1], F16, tag="cmax16")
            cm1 = singles.tile([1, 1], F32, tag="cm1")
            cm_all = singles.tile([128, 1], F32, tag="cm_all")
            idot = singles.tile([128, NI], F32, tag="idot")
            maxP = singles.tile([128, NI], F32, tag="maxP")
            k116 = singles.tile([128, 1], F16, tag="k116")
            k11 = singles.tile([1, 1], F32, tag="k11")
            negk = singles.tile([128, 1], F32, tag="negk")
            e2 = singles.tile([128, NI], F32, tag="e2")
            u_t = singles.tile([128, NI], F32, tag="u_t")
            su1 = singles.tile([128, 1], F32, tag="su1")
            su_all = singles.tile([128, 1], F32, tag="su_all")
            rec2 = singles.tile([128, 1], F32, tag="rec2")
            wt2 = singles.tile([128, NI], F16, tag="wt2")
            o2b16 = singles.tile([128, d], F16, tag="o2b16")
            ones32 = singles.tile([128, 1], F32, tag="ones32")
            wz = singles.tile([128, 128], F16, tag="wz")

            maug_r = maug_s[:].rearrange("p (c x) -> p c x", x=D1)

            # ==== loads: 5 packed transfers, one ring, priority order ====
            nc.sync.dma_start(out=g1_s, in_=g1_d[:, :])
            for kc in range(2):  # xt per kc-half; kc0 matmuls start first
                nc.sync.dma_start(out=xt_s[:, kc * Li:(kc + 1) * Li],
                                  in_=xt_d[:, kc * Li:(kc + 1) * Li])
            nc.sync.dma_start(out=maug_s, in_=maug_d[:, :])
            nc.sync.dma_start(out=g4_s, in_=g4_d[:, :])

            nc.vector.memset(wz, 0.0)
            nc.vector.memset(ones32, 1.0)
            # load the ACT exp table during the load phase
            actwarm = scr.tile([1, 1], F32, tag="actw")
            nc.scalar.activation(out=actwarm, in_=ones32[0:1, 0:1], func=ACTF.Exp)

            # ==== PE warmup on the memset tile: no DMA dependency at all ====
            for w in range(16):
                psw = po.tile([128, 128], F32, tag="po")
                nc.tensor.matmul(psw, wz, wz, start=True, stop=True)

            # broadcast [1,1] -> [128,1] without gpsimd: replicate along the
            # free dim on DVE (stride-0 read), then transpose on the PE
            def bcast_col(dst_sb, src11):
                row = scr.tile([1, 128], F16, tag="brow")
                nc.vector.tensor_copy(row, src11[0:1, 0:1].broadcast_to([1, 128]))
                ps_b = po.tile([128, 1], F16, tag="po")
                nc.tensor.transpose(ps_b, row, ident16[0:1, 0:1])
                nc.vector.tensor_copy(dst_sb, ps_b)

            # ==== DVE preprocessing ====
            # fold dot_scale into x^T (per-partition scalar, in place)
            dsc32 = scr.tile([128, 2], F32, tag="dsc32")
            nc.vector.tensor_copy(dsc32, dsc_s)
            for kc in range(2):
                nc.vector.tensor_scalar_mul(
                    xts[:, kc * Li:(kc + 1) * Li],
                    xt_s[:, kc * Li:(kc + 1) * Li], dsc32[:, kc:kc + 1])
            # mvec[j] = m[j,:] . w_mem on the PE (tiny matmuls vs w_mem column)
            psum_mv = po.tile([128, NJ], F32, tag="po")
            for jc in range(NJ):
                for kc in range(2):
                    nc.tensor.matmul(
                        psum_mv[:, jc:jc + 1],
                        mt_s[:, kc * Lmp + jc * 128: kc * Lmp + (jc + 1) * 128],
                        wmemc[:, kc:kc + 1],
                        start=(kc == 0), stop=(kc == 1))
            nc.vector.tensor_add(mvec, psum_mv, mp_s)
            nc.vector.reduce_max(out=cmax16, in_=mvec, axis=AX.X)
            ps_c = po.tile([1, 128], F16, tag="po")
            nc.tensor.transpose(ps_c, cmax16, ident16)
            nc.vector.reduce_max(out=cm1, in_=ps_c, axis=AX.X)
            bcast_col(cm_all, cm1)
            nc.vector.tensor_scalar(
                out=bias_sb, in0=mvec, scalar1=cm_all[:, 0:1], scalar2=-4.0,
                op0=ALU.subtract, op1=ALU.add)

            # ==== phase 1 group: scores + exp + running max for (h, jc) ====
            def ph1_group(h, jc):
                psum_s = ps.tile([128, H], F32, tag="ps")
                for kc in range(2):
                    for bs in range(0, H, 512):  # fp16 moving operand max 512
                        nc.tensor.matmul(
                            psum_s[:, bs:bs + 512],
                            mt_s[:, kc * Lmp + jc * 128: kc * Lmp + (jc + 1) * 128],
                            xts[:, kc * Li + h * H + bs: kc * Li + h * H + bs + 512],
                            start=(kc == 0), stop=(kc == 1))
                pt_sl = PT[:, jc * Li + h * H: jc * Li + (h + 1) * H]
                nc.scalar.activation(out=pt_sl, in_=psum_s, func=ACTF.Exp,
                                     bias=bias_sb[:, jc:jc + 1], scale=1.0)
                m_sl = M1[:, h * H:(h + 1) * H]
                if jc == 0:
                    nc.vector.tensor_copy(m_sl, pt_sl)
                else:
                    nc.vector.tensor_max(m_sl, m_sl, pt_sl)

            # ==== phase 2 group + epilogue for i-tile it ====
            def ph2_group(it):
                psum_o = po.tile([128, D1], F32, tag="po")
                for jc in range(NJ):
                    nc.tensor.matmul(
                        psum_o,
                        PT[:, jc * Li + it * 128: jc * Li + (it + 1) * 128],
                        maug_r[:, jc, :],
                        start=(jc == 0), stop=(jc == NJ - 1))
                rec_s = scr.tile([128, 1], F32, tag="rec_s")
                nc.vector.reciprocal(rec_s, psum_o[:, d:d + 1])
                o1_sl = O1_all[:, it * d:(it + 1) * d]
                nc.scalar.activation(out=o1_sl, in_=psum_o[:, 0:d],
                                     func=ACTF.Copy, scale=rec_s[:, 0:1])

            def b2_batch(q):  # x*o1 for i-tiles [q*QT, (q+1)*QT)
                sl = slice(q * QT * d, (q + 1) * QT * d)
                nc.vector.tensor_mul(B2_all[:, sl], O1_all[:, sl], xb_s[:, sl])

            def store(src, col0, t0, nt=QT):
                nc.sync.dma_start(
                    out=out_d[t0 * 128:(t0 + nt) * 128,
                              col0:col0 + d].rearrange("(c p) x -> p c x", p=128),
                    in_=src[:, t0 * d:(t0 + nt) * d].rearrange(
                        "p (c x) -> p c x", x=d))

            # ---- phase 1 h=0 ----
            for jc in range(NJ):
                ph1_group(0, jc)

            # ---- phase 1 h=1 with phase-2 h=0 interleaved ----
            ph1_group(1, 0)
            ph1_group(1, 1)
            for jc in range(2, NJ):
                ph1_group(1, jc)
                ph2_group(jc - 2)          # it 0..6
            # idot[i] = x[i,:] . w_in on the PE: raw x^T tiles against the
            # w_in column (fills PE stalls during the exp stream; keeps the
            # DVE queue free of load-gated work)
            psum_id = po.tile([128, NI], F32, tag="po")
            for it in range(NI):
                for kc in range(2):
                    nc.tensor.matmul(
                        psum_id[:, it:it + 1],
                        xt_s[:, kc * Li + it * 128: kc * Li + (it + 1) * 128],
                        winc[:, kc:kc + 1],
                        start=(kc == 0), stop=(kc == 1))
            nc.vector.tensor_copy(idot, psum_id)
            # k chain for idot max (tiny, early)
            nc.vector.reduce_max(out=k116, in_=idot, axis=AX.X)
            ps_k = po.tile([1, 128], F16, tag="po")
            nc.tensor.transpose(ps_k, k116, ident16)
            nc.vector.reduce_max(out=k11, in_=ps_k, axis=AX.X)
            bcast_col(negk, k11)
            nc.vector.tensor_scalar_mul(negk, negk, -1.0)
            ph2_group(7)
            b2_batch(0)
            b2_batch(1)
            store(O1_all, 0, 0)
            store(B2_all, d, 0)
            store(O1_all, 0, QT)
            store(B2_all, d, QT)

            # ---- stage C: maxP[i] = max over partitions of M1 (4-batched) ----
            for t0 in range(0, NI, 4):
                psT = po.tile([128, 512], F16, tag="po")
                for t in range(4):
                    nc.tensor.transpose(
                        psT[:, t * 128:(t + 1) * 128],
                        M1[:, (t0 + t) * 128:(t0 + t + 1) * 128], ident16)
                nc.vector.reduce_max(
                    out=maxP[:, t0:t0 + 4],
                    in_=psT[:].rearrange("p (c x) -> p c x", x=128), axis=AX.X)

            # ---- stage D: weight_two -> o2b ----
            ph2_group(8)
            ph2_group(9)
            nc.scalar.activation(out=e2, in_=idot, func=ACTF.Exp,
                                 bias=negk[:, 0:1], scale=1.0)
            nc.vector.tensor_mul(u_t, maxP, e2)
            nc.vector.reduce_sum(out=su1, in_=u_t, axis=AX.X)
            ph2_group(10)
            # su_all[p] = sum_k su1[k] for every p: stride-0 stationary matmul
            ps_su = po.tile([128, 1], F32, tag="po")
            nc.tensor.matmul(ps_su, su1[:, 0:1].broadcast_to([128, 128]),
                             ones32, start=True, stop=True)
            nc.vector.tensor_copy(su_all, ps_su)
            nc.vector.reciprocal(rec2, su_all)
            nc.vector.tensor_scalar(out=wt2, in0=u_t, scalar1=rec2[:, 0:1],
                                    scalar2=None, op0=ALU.mult)
            ph2_group(11)
            # o2 replicated into all partitions directly: stationary is the
            # wt2 column broadcast along its free dim
            ps_o2b = po.tile([128, d], F32, tag="po")
            for ic in range(NI):
                nc.tensor.matmul(ps_o2b,
                                 wt2[:, ic:ic + 1].broadcast_to([128, 128]),
                                 xb_s[:, ic * d:(ic + 1) * d],
                                 start=(ic == 0), stop=(ic == NI - 1))
            nc.vector.tensor_copy(o2b16, ps_o2b)
            # block 3 for h=0 (o2b known; overlaps remaining phase 2)
            o2_bc0 = o2b16[:].unsqueeze(1).broadcast_to([128, TPH, d])
            nc.vector.tensor_mul(
                B3_all[:, 0:TPH * d].rearrange("p (c x) -> p c x", x=d),
                O1_all[:, 0:TPH * d].rearrange("p (c x) -> p c x", x=d), o2_bc0)
            store(B3_all, 2 * d, 0)
            store(B3_all, 2 * d, QT)
            for it in range(12, NI):
                ph2_group(it)
            b2_batch(2)
            store(O1_all, 0, 2 * QT)
            store(B2_all, d, 2 * QT)
            o2_bc1 = o2b16[:].unsqueeze(1).broadcast_to([128, QT, d])
            nc.vector.tensor_mul(
                B3_all[:, 8 * d:12 * d].rearrange("p (c x) -> p c x", x=d),
                O1_all[:, 8 * d:12 * d].rearrange("p (c x) -> p c x", x=d),
                o2_bc1)
            store(B3_all, 2 * d, 2 * QT)
            # last quarter in two halves to shorten the store tail
            b2_batch(3)
            o2_bc2 = o2b16[:].unsqueeze(1).broadcast_to([128, 2, d])
            nc.vector.tensor_mul(
                B3_all[:, 12 * d:14 * d].rearrange("p (c x) -> p c x", x=d),
                O1_all[:, 12 * d:14 * d].rearrange("p (c x) -> p c x", x=d),
                o2_bc2)
            store(O1_all, 0, 12, 2)
            store(B2_all, d, 12, 2)
            store(B3_all, 2 * d, 12, 2)
            nc.vector.tensor_mul(
                B3_all[:, 14 * d:16 * d].rearrange("p (c x) -> p c x", x=d),
                O1_all[:, 14 * d:16 * d].rearrange("p (c x) -> p c x", x=d),
                o2_bc2)
            store(O1_all, 0, 14, 2)
            store(B2_all, d, 14, 2)
            store(B3_all, 2 * d, 14, 2)

    nc.compile()
    return nc


def _prep_core_inputs(x_b, m_b, mask_b, w_in, w_mem, dsc, Lmp):
    """Host-side shard prep: permute unmasked memory rows first, pad to Lmp,
    and marshal operands into the exact on-chip layouts (transpose / fp16 cast /
    constant padding only — all arithmetic happens on device)."""
    Li, d = x_b.shape
    NI, NJ, D1 = Li // 128, Lmp // 128, d + 1
    idx = np.flatnonzero(mask_b != 0)
    cnt = len(idx)
    m_p = np.zeros((Lmp, d), dtype=np.float32)
    m_p[:cnt] = m_b[idx]
    x16 = x_b.astype(np.float16)
    m16 = m_p.astype(np.float16)
    xt = np.ascontiguousarray(
        x16.T.reshape(2, 128, Li).transpose(1, 0, 2).reshape(128, 2 * Li))
    mt = m16.T.reshape(2, 128, Lmp).transpose(1, 0, 2).reshape(128, 2 * Lmp)
    maug = np.ones((Lmp, D1), dtype=np.float16)
    maug[:, :d] = m16
    maug = np.ascontiguousarray(
        maug.reshape(NJ, 128, D1).transpose(1, 0, 2).reshape(128, NJ * D1))
    # g1: ident | dsc(f16) | wmemc | winc | mp | mt
    flat = np.zeros(Lmp, dtype=np.float16)
    flat[cnt:] = np.float16(-65504.0)  # exp of the f16-lowest pad underflows to 0
    g1 = np.empty((128, 128 + 2 + 2 + 2 + NJ + 2 * Lmp), dtype=np.float16)
    g1[:, 0:128] = np.eye(128, dtype=np.float16)
    g1[:, 128:130] = dsc.astype(np.float16).reshape(2, 128).T
    g1[:, 130:132] = w_mem.astype(np.float16).reshape(2, 128).T
    g1[:, 132:134] = w_in.astype(np.float16).reshape(2, 128).T
    g1[:, 134:134 + NJ] = flat.reshape(NJ, 128).T
    g1[:, 134 + NJ:] = mt
    # g4: xb
    g4 = np.ascontiguousarray(
        x16.reshape(NI, 128, d).transpose(1, 0, 2).reshape(128, NI * d))
    return {"g1": np.ascontiguousarray(g1), "xt": xt,
            "maug": maug, "g4": g4}


def kernel(input, memory, mask, w_in, w_mem, dot_scale, _tmpdir=None):
    global LAST_RESULTS
    input = np.asarray(input, dtype=np.float32)
    memory = np.asarray(memory, dtype=np.float32)
    mask = np.asarray(mask)
    w_in = np.asarray(w_in, dtype=np.float32)
    w_mem = np.asarray(w_mem, dtype=np.float32)
    dot_scale = np.asarray(dot_scale, dtype=np.float32)

    bsz, Li, d = input.shape
    assert bsz == N_CORES

    counts = [int((mask[b] != 0).sum()) for b in range(bsz)]
    Lmp = max(128, int(math.ceil(max(counts) / 128.0)) * 128)

    key = (Li, Lmp, d)
    if key not in _NC_CACHE:
        _NC_CACHE[key] = build_nc(Li, Lmp, d)
    nc = _NC_CACHE[key]

    in_maps = [
        _prep_core_inputs(input[b], memory[b], mask[b], w_in, w_mem, dot_scale, Lmp)
        for b in range(bsz)
    ]
    res = run_bass_kernel_spmd(nc, in_maps, list(range(N_CORES)), tmpdir=_tmpdir)
    LAST_RESULTS = res
    out = np.empty((bsz, Li, 4 * d), dtype=np.float32)
    out[:, :, 0:d] = input
    for b in range(bsz):
        out[b, :, d:4 * d] = res.results[b]["out"].astype(np.float32)
    return out


# revision 49
# speedup vs baseline: 1.3076x; 1.0542x over previous
"""BiAttention Trainium2 Bass kernel (v9 — fp16 streaming pipeline).

Per-core (one batch per NeuronCore, batch=8 over 8 cores):
  att[i,j] = input_dot[i] + memory_dot[j] + (input*dot_scale) @ memory^T - NEG*(1-mask[j])
  weight_one = softmax_j(att);  output_one = weight_one @ memory
  weight_two = softmax_i(max_j att);  output_two = weight_two @ input
  out = concat([input, output_one, input*output_one, output_two*output_one], -1)

Implementation notes:
  - input_dot cancels in softmax_j; only memory_dot + mask pad enter the bias.
  - Unmasked memory rows are permuted first host-side; only Lmp rows computed.
  - Scores built transposed (S^T[j,i]): per-j bias is a per-partition ACT bias,
    exp lands in the P^T layout phase 2 needs.  C = max(mvec)+4 global shift.
  - Everything on-chip is fp16 (PT, operands, outputs); fp32 psum/stats.
  - dot_scale folded into x^T on device; mvec via tiny PE matmuls vs a w_mem
    column; idot via chunked DVE mul+reduce.
  - Cross-partition max/sum/broadcast via PE transpose + ones-row matmuls —
    gpsimd is left completely idle (its Q7 custom-op library load costs ~6us
    behind the load queue, and TensorTensor there forces a lib switch).
  - Loads: 5 packed fp16 transfers on one ring in priority order (smalls ride
    inside the first transfer; per-transfer fixed cost dominates small DMAs).
  - Device writes only the 3 computed output blocks in fp16; the `input` block
    is assembled host-side (pure copy).  Host prep is layout/dtype marshalling
    only — all arithmetic happens on device.
  - PE warmup matmuls on a memset tile un-throttle the HAM clock gate before
    any DMA lands.
"""

import math
import numpy as np

import concourse.bass as bass
import concourse.mybir as mybir
import concourse.tile as tile
import concourse.bacc as bacc
from concourse import bass_isa
from concourse.bass_utils import run_bass_kernel_spmd

F32 = mybir.dt.float32
F16 = mybir.dt.float16
AX = mybir.AxisListType
ALU = mybir.AluOpType
ACTF = mybir.ActivationFunctionType

N_CORES = 8
NEG = 1e30

_NC_CACHE: dict = {}
LAST_RESULTS = None  # BassKernelResults of the most recent run (for test harness)


def build_nc(Li: int, Lmp: int, d: int):
    """Build the single-core SPMD program.  Li, d fixed; Lmp = padded #unmasked."""
    assert Li % 128 == 0 and Lmp % 128 == 0 and d == 256
    NI = Li // 128          # i tiles (16)
    NJ = Lmp // 128         # j tiles (9)
    D1 = d + 1
    H = 1024                # phase-1 i-chunk
    NH = Li // H            # 2
    TPH = H // 128          # i tiles per chunk (8)
    QT = 4                  # i tiles per output store chunk

    # packed g1: ident16 | dsc(f16) | wmemc | winc | mp | mt
    G1W = 128 + 2 + 2 + 2 + NJ + 2 * Lmp
    # packed g4: xb | winb
    G4W = NI * d + d

    nc = bacc.Bacc("TRN2", target_bir_lowering=False, debug=False,
                   num_devices=N_CORES)

    g1_d = nc.dram_tensor("g1", [128, G1W], F16, kind="ExternalInput")
    xt_d = nc.dram_tensor("xt", [128, 2 * Li], F16, kind="ExternalInput")
    maug_d = nc.dram_tensor("maug", [128, NJ * D1], F16, kind="ExternalInput")
    g4_d = nc.dram_tensor("g4", [128, G4W], F16, kind="ExternalInput")
    out_d = nc.dram_tensor("out", [Li, 3 * d], F16, kind="ExternalOutput")

    with tile.TileContext(nc) as tc:
        with (
            tc.tile_pool(name="singles", bufs=1) as singles,
            tc.tile_pool(name="scr", bufs=2) as scr,
            tc.tile_pool(name="ps", bufs=2, space="PSUM") as ps,
            tc.tile_pool(name="po", bufs=4, space="PSUM") as po,
        ):
            # ---- resident tiles ----
            g1_s = singles.tile([128, G1W], F16, tag="g1_s")
            xt_s = singles.tile([128, 2 * Li], F16, tag="xt_s")
            xts = singles.tile([128, 2 * Li], F16, tag="xts")
            maug_s = singles.tile([128, NJ * D1], F16, tag="maug_s")
            g4_s = singles.tile([128, G4W], F16, tag="g4_s")

            ident16 = g1_s[:, 0:128]
            dsc_s = g1_s[:, 128:130]
            wmemc = g1_s[:, 130:132]
            winc = g1_s[:, 132:134]
            mp_s = g1_s[:, 134:134 + NJ]
            mt_s = g1_s[:, 134 + NJ:134 + NJ + 2 * Lmp]
            xb_s = g4_s[:, 0:NI * d]
            winb_s = g4_s[:, NI * d:NI * d + d]

            PT = singles.tile([128, NJ * Li], F16, tag="PT")
            M1 = singles.tile([128, Li], F16, tag="M1")
            O1_all = singles.tile([128, NI * d], F16, tag="O1_all")
            B2_all = singles.tile([128, NI * d], F16, tag="B2_all")
            B3_all = singles.tile([128, NI * d], F16, tag="B3_all")
            xscr = singles.tile([128, NI * d], F16, tag="xscr")

            mvec = singles.tile([128, NJ], F32, tag="mvec")
            bias_sb = singles.tile([128, NJ], F32, tag="bias_sb")
            cmax16 = singles.tile([128, 1], F16, tag="cmax16")
            cm1 = singles.tile([1, 1], F32, tag="cm1")
            cm_all = singles.tile([128, 1], F32, tag="cm_all")
            idot = singles.tile([128, NI], F32, tag="idot")
            maxP = singles.tile([128, NI], F32, tag="maxP")
            k116 = singles.tile([128, 1], F16, tag="k116")
            k11 = singles.tile([1, 1], F32, tag="k11")
            negk = singles.tile([128, 1], F32, tag="negk")
            e2 = singles.tile([128, NI], F32, tag="e2")
            u_t = singles.tile([128, NI], F32, tag="u_t")
            su1 = singles.tile([128, 1], F32, tag="su1")
            su_all = singles.tile([128, 1], F32, tag="su_all")
            rec2 = singles.tile([128, 1], F32, tag="rec2")
            wt2 = singles.tile([128, NI], F16, tag="wt2")
            o2b16 = singles.tile([128, d], F16, tag="o2b16")
            ones32 = singles.tile([128, 1], F32, tag="ones32")
            wz = singles.tile([128, 128], F16, tag="wz")

            maug_r = maug_s[:].rearrange("p (c x) -> p c x", x=D1)

            # ==== loads: 5 packed transfers, one ring, priority order ====
            nc.sync.dma_start(out=g1_s, in_=g1_d[:, :])
            for kc in range(2):  # xt per kc-half; kc0 matmuls start first
                nc.sync.dma_start(out=xt_s[:, kc * Li:(kc + 1) * Li],
                                  in_=xt_d[:, kc * Li:(kc + 1) * Li])
            nc.sync.dma_start(out=maug_s, in_=maug_d[:, :])
            nc.sync.dma_start(out=g4_s, in_=g4_d[:, :])

            nc.vector.memset(wz, 0.0)
            nc.vector.memset(ones32, 1.0)
            # load the ACT exp table during the load phase
            actwarm = scr.tile([1, 1], F32, tag="actw")
            nc.scalar.activation(out=actwarm, in_=ones32[0:1, 0:1], func=ACTF.Exp)

            # ==== PE warmup on the memset tile: no DMA dependency at all ====
            for w in range(24):
                psw = po.tile([128, 128], F32, tag="po")
                nc.tensor.matmul(psw, wz, wz, start=True, stop=True)

            # broadcast [1,1] -> [128,1] without gpsimd: replicate along the
            # free dim on DVE (stride-0 read), then transpose on the PE
            def bcast_col(dst_sb, src11):
                row = scr.tile([1, 128], F16, tag="brow")
                nc.vector.tensor_copy(row, src11[0:1, 0:1].broadcast_to([1, 128]))
                ps_b = po.tile([128, 1], F16, tag="po")
                nc.tensor.transpose(ps_b, row, ident16[0:1, 0:1])
                nc.vector.tensor_copy(dst_sb, ps_b)

            # ==== DVE preprocessing ====
            # fold dot_scale into x^T (per-partition scalar, in place)
            dsc32 = scr.tile([128, 2], F32, tag="dsc32")
            nc.vector.tensor_copy(dsc32, dsc_s)
            for kc in range(2):
                nc.vector.tensor_scalar_mul(
                    xts[:, kc * Li:(kc + 1) * Li],
                    xt_s[:, kc * Li:(kc + 1) * Li], dsc32[:, kc:kc + 1])
            # mvec[j] = m[j,:] . w_mem on the PE (tiny matmuls vs w_mem column)
            psum_mv = po.tile([128, NJ], F32, tag="po")
            for jc in range(NJ):
                for kc in range(2):
                    nc.tensor.matmul(
                        psum_mv[:, jc:jc + 1],
                        mt_s[:, kc * Lmp + jc * 128: kc * Lmp + (jc + 1) * 128],
                        wmemc[:, kc:kc + 1],
                        start=(kc == 0), stop=(kc == 1))
            nc.vector.tensor_add(mvec, psum_mv, mp_s)
            nc.vector.reduce_max(out=cmax16, in_=mvec, axis=AX.X)
            ps_c = po.tile([1, 128], F16, tag="po")
            nc.tensor.transpose(ps_c, cmax16, ident16)
            nc.vector.reduce_max(out=cm1, in_=ps_c, axis=AX.X)
            bcast_col(cm_all, cm1)
            nc.vector.tensor_scalar(
                out=bias_sb, in0=mvec, scalar1=cm_all[:, 0:1], scalar2=-4.0,
                op0=ALU.subtract, op1=ALU.add)

            # ==== phase 1 group: scores + exp + running max for (h, jc) ====
            def ph1_group(h, jc):
                psum_s = ps.tile([128, H], F32, tag="ps")
                for kc in range(2):
                    for bs in range(0, H, 512):  # fp16 moving operand max 512
                        nc.tensor.matmul(
                            psum_s[:, bs:bs + 512],
                            mt_s[:, kc * Lmp + jc * 128: kc * Lmp + (jc + 1) * 128],
                            xts[:, kc * Li + h * H + bs: kc * Li + h * H + bs + 512],
                            start=(kc == 0), stop=(kc == 1))
                pt_sl = PT[:, jc * Li + h * H: jc * Li + (h + 1) * H]
                nc.scalar.activation(out=pt_sl, in_=psum_s, func=ACTF.Exp,
                                     bias=bias_sb[:, jc:jc + 1], scale=1.0)
                m_sl = M1[:, h * H:(h + 1) * H]
                if jc == 0:
                    nc.vector.tensor_copy(m_sl, pt_sl)
                else:
                    nc.vector.tensor_max(m_sl, m_sl, pt_sl)

            # ==== phase 2 group + epilogue for i-tile it ====
            def ph2_group(it):
                psum_o = po.tile([128, D1], F32, tag="po")
                for jc in range(NJ):
                    nc.tensor.matmul(
                        psum_o,
                        PT[:, jc * Li + it * 128: jc * Li + (it + 1) * 128],
                        maug_r[:, jc, :],
                        start=(jc == 0), stop=(jc == NJ - 1))
                rec_s = scr.tile([128, 1], F32, tag="rec_s")
                nc.vector.reciprocal(rec_s, psum_o[:, d:d + 1])
                o1_sl = O1_all[:, it * d:(it + 1) * d]
                nc.scalar.activation(out=o1_sl, in_=psum_o[:, 0:d],
                                     func=ACTF.Copy, scale=rec_s[:, 0:1])

            def b2_batch(q):  # x*o1 for i-tiles [q*QT, (q+1)*QT)
                sl = slice(q * QT * d, (q + 1) * QT * d)
                nc.vector.tensor_mul(B2_all[:, sl], O1_all[:, sl], xb_s[:, sl])

            def store(src, col0, t0, nt=QT, eng=None):
                (eng or nc.sync).dma_start(
                    out=out_d[t0 * 128:(t0 + nt) * 128,
                              col0:col0 + d].rearrange("(c p) x -> p c x", p=128),
                    in_=src[:, t0 * d:(t0 + nt) * d].rearrange(
                        "p (c x) -> p c x", x=d))

            # ---- phase 1 h=0 ----
            for jc in range(NJ):
                ph1_group(0, jc)

            # ---- phase 1 h=1 with phase-2 h=0 interleaved ----
            ph1_group(1, 0)
            ph1_group(1, 1)
            for jc in range(2, NJ):
                ph1_group(1, jc)
                ph2_group(jc - 2)          # it 0..6
            # idot[i] = x[i,:] . w_in  (DVE, chunked; all bias-path deps
            # land in g1 so these can no longer poison the DVE queue)
            for q in range(NI // QT):
                sl = slice(q * QT * d, (q + 1) * QT * d)
                nc.vector.tensor_mul(
                    xscr[:, sl].rearrange("p (c x) -> p c x", x=d),
                    xb_s[:, sl].rearrange("p (c x) -> p c x", x=d),
                    winb_s[:].unsqueeze(1).broadcast_to([128, QT, d]))
                nc.vector.reduce_sum(
                    out=idot[:, q * QT:(q + 1) * QT],
                    in_=xscr[:, sl].rearrange("p (c x) -> p c x", x=d),
                    axis=AX.X)
            # k chain for idot max (tiny, early)
            nc.vector.reduce_max(out=k116, in_=idot, axis=AX.X)
            ps_k = po.tile([1, 128], F16, tag="po")
            nc.tensor.transpose(ps_k, k116, ident16)
            nc.vector.reduce_max(out=k11, in_=ps_k, axis=AX.X)
            bcast_col(negk, k11)
            nc.vector.tensor_scalar_mul(negk, negk, -1.0)
            ph2_group(7)
            b2_batch(0)
            b2_batch(1)
            store(O1_all, 0, 0)
            store(B2_all, d, 0)
            store(O1_all, 0, QT)
            store(B2_all, d, QT)

            # ---- stage C: maxP[i] = max over partitions of M1 (4-batched) ----
            for t0 in range(0, NI, 4):
                psT = po.tile([128, 512], F16, tag="po")
                for t in range(4):
                    nc.tensor.transpose(
                        psT[:, t * 128:(t + 1) * 128],
                        M1[:, (t0 + t) * 128:(t0 + t + 1) * 128], ident16)
                nc.vector.reduce_max(
                    out=maxP[:, t0:t0 + 4],
                    in_=psT[:].rearrange("p (c x) -> p c x", x=128), axis=AX.X)

            # ---- stage D: weight_two -> o2b ----
            ph2_group(8)
            ph2_group(9)
            nc.scalar.activation(out=e2, in_=idot, func=ACTF.Exp,
                                 bias=negk[:, 0:1], scale=1.0)
            nc.vector.tensor_mul(u_t, maxP, e2)
            nc.vector.reduce_sum(out=su1, in_=u_t, axis=AX.X)
            ph2_group(10)
            # su_all[p] = sum_k su1[k] for every p: stride-0 stationary matmul
            ps_su = po.tile([128, 1], F32, tag="po")
            nc.tensor.matmul(ps_su, su1[:, 0:1].broadcast_to([128, 128]),
                             ones32, start=True, stop=True)
            nc.vector.tensor_copy(su_all, ps_su)
            nc.vector.reciprocal(rec2, su_all)
            nc.vector.tensor_scalar(out=wt2, in0=u_t, scalar1=rec2[:, 0:1],
                                    scalar2=None, op0=ALU.mult)
            ph2_group(11)
            # o2 replicated into all partitions directly: stationary is the
            # wt2 column broadcast along its free dim
            ps_o2b = po.tile([128, d], F32, tag="po")
            for ic in range(NI):
                nc.tensor.matmul(ps_o2b,
                                 wt2[:, ic:ic + 1].broadcast_to([128, 128]),
                                 xb_s[:, ic * d:(ic + 1) * d],
                                 start=(ic == 0), stop=(ic == NI - 1))
            nc.vector.tensor_copy(o2b16, ps_o2b)
            # block 3 for h=0 (o2b known; overlaps remaining phase 2)
            o2_bc0 = o2b16[:].unsqueeze(1).broadcast_to([128, TPH, d])
            nc.vector.tensor_mul(
                B3_all[:, 0:TPH * d].rearrange("p (c x) -> p c x", x=d),
                O1_all[:, 0:TPH * d].rearrange("p (c x) -> p c x", x=d), o2_bc0)
            store(B3_all, 2 * d, 0, eng=nc.scalar)
            store(B3_all, 2 * d, QT, eng=nc.scalar)
            for it in range(12, NI):
                ph2_group(it)
            b2_batch(2)
            store(O1_all, 0, 2 * QT)
            store(B2_all, d, 2 * QT)
            o2_bc1 = o2b16[:].unsqueeze(1).broadcast_to([128, QT, d])
            nc.vector.tensor_mul(
                B3_all[:, 8 * d:12 * d].rearrange("p (c x) -> p c x", x=d),
                O1_all[:, 8 * d:12 * d].rearrange("p (c x) -> p c x", x=d),
                o2_bc1)
            store(B3_all, 2 * d, 2 * QT, eng=nc.scalar)
            # last quarter in two halves to shorten the store tail
            b2_batch(3)
            o2_bc2 = o2b16[:].unsqueeze(1).broadcast_to([128, 2, d])
            nc.vector.tensor_mul(
                B3_all[:, 12 * d:14 * d].rearrange("p (c x) -> p c x", x=d),
                O1_all[:, 12 * d:14 * d].rearrange("p (c x) -> p c x", x=d),
                o2_bc2)
            store(O1_all, 0, 12, 2)
            store(B2_all, d, 12, 2)
            store(B3_all, 2 * d, 12, 2, eng=nc.scalar)
            nc.vector.tensor_mul(
                B3_all[:, 14 * d:16 * d].rearrange("p (c x) -> p c x", x=d),
                O1_all[:, 14 * d:16 * d].rearrange("p (c x) -> p c x", x=d),
                o2_bc2)
            store(O1_all, 0, 14, 2)
            store(B2_all, d, 14, 2)
            store(B3_all, 2 * d, 14, 2, eng=nc.scalar)

    nc.compile()
    return nc


def _prep_core_inputs(x_b, m_b, mask_b, w_in, w_mem, dsc, Lmp):
    """Host-side shard prep: permute unmasked memory rows first, pad to Lmp,
    and marshal operands into the exact on-chip layouts (transpose / fp16 cast /
    constant padding only — all arithmetic happens on device)."""
    Li, d = x_b.shape
    NI, NJ, D1 = Li // 128, Lmp // 128, d + 1
    idx = np.flatnonzero(mask_b != 0)
    cnt = len(idx)
    m_p = np.zeros((Lmp, d), dtype=np.float32)
    m_p[:cnt] = m_b[idx]
    x16 = x_b.astype(np.float16)
    m16 = m_p.astype(np.float16)
    xt = np.ascontiguousarray(
        x16.T.reshape(2, 128, Li).transpose(1, 0, 2).reshape(128, 2 * Li))
    mt = m16.T.reshape(2, 128, Lmp).transpose(1, 0, 2).reshape(128, 2 * Lmp)
    maug = np.ones((Lmp, D1), dtype=np.float16)
    maug[:, :d] = m16
    maug = np.ascontiguousarray(
        maug.reshape(NJ, 128, D1).transpose(1, 0, 2).reshape(128, NJ * D1))
    # g1: ident | dsc(f16) | wmemc | winc | mp | mt
    flat = np.zeros(Lmp, dtype=np.float16)
    flat[cnt:] = np.float16(-65504.0)  # exp of the f16-lowest pad underflows to 0
    g1 = np.empty((128, 128 + 2 + 2 + 2 + NJ + 2 * Lmp), dtype=np.float16)
    g1[:, 0:128] = np.eye(128, dtype=np.float16)
    g1[:, 128:130] = dsc.astype(np.float16).reshape(2, 128).T
    g1[:, 130:132] = w_mem.astype(np.float16).reshape(2, 128).T
    g1[:, 132:134] = w_in.astype(np.float16).reshape(2, 128).T
    g1[:, 134:134 + NJ] = flat.reshape(NJ, 128).T
    g1[:, 134 + NJ:] = mt
    # g4: xb | winb
    g4 = np.empty((128, NI * d + d), dtype=np.float16)
    g4[:, 0:NI * d] = x16.reshape(NI, 128, d).transpose(1, 0, 2).reshape(
        128, NI * d)
    g4[:, NI * d:] = np.broadcast_to(w_in.astype(np.float16)[None, :], (128, d))
    return {"g1": np.ascontiguousarray(g1), "xt": xt,
            "maug": maug, "g4": g4}


def kernel(input, memory, mask, w_in, w_mem, dot_scale, _tmpdir=None):
    global LAST_RESULTS
    input = np.asarray(input, dtype=np.float32)
    memory = np.asarray(memory, dtype=np.float32)
    mask = np.asarray(mask)
    w_in = np.asarray(w_in, dtype=np.float32)
    w_mem = np.asarray(w_mem, dtype=np.float32)
    dot_scale = np.asarray(dot_scale, dtype=np.float32)

    bsz, Li, d = input.shape
    assert bsz == N_CORES

    counts = [int((mask[b] != 0).sum()) for b in range(bsz)]
    Lmp = max(128, int(math.ceil(max(counts) / 128.0)) * 128)

    key = (Li, Lmp, d)
    if key not in _NC_CACHE:
        _NC_CACHE[key] = build_nc(Li, Lmp, d)
    nc = _NC_CACHE[key]

    in_maps = [
        _prep_core_inputs(input[b], memory[b], mask[b], w_in, w_mem, dot_scale, Lmp)
        for b in range(bsz)
    ]
    res = run_bass_kernel_spmd(nc, in_maps, list(range(N_CORES)), tmpdir=_tmpdir)
    LAST_RESULTS = res
    out = np.empty((bsz, Li, 4 * d), dtype=np.float32)
    out[:, :, 0:d] = input
    for b in range(bsz):
        out[b, :, d:4 * d] = res.results[b]["out"].astype(np.float32)
    return out
